# revision 15
# baseline (speedup 1.0000x reference)
"""Transformer block (LN->MHA->residual->LN->MLP->residual) on 8 trn2 cores.

Data-parallel over batch: each of the 8 NeuronCores processes one [1024, 768]
batch element with the full weight set.  No collectives.

Tuned from the 467us baseline via trace analysis (final ~335-355us):
  - token-major for LN stats/residuals/output; feature-major (PE transposes)
    for matmul operands; one [P,1024] fp32 PSUM tag shared by every matmul
    phase so no pool boundary (= no PE drain stall) between attention, proj
    and MLP; attnv gets a private 2-buf PSUM pool.
  - x loaded as paired-tile DMAs on the sync queue ahead of everything
    (single-ring DMA triggers serialize ~2.6us/tile); w_qk transfers paced
    behind x via a gpsimd data dependency; w_fc1/w_fc2 loaded during proj.
  - column biases (b_qk, b_fc1) loaded as [m,128] rows and PE-transposed
    (a 4-byte-element gather DMA costs >20us); row biases applied via K=1
    ones-row matmul PSUM preload (v) or pre-added into the residual on
    idle DVE cycles (b_proj, b_fc2, in-place in x_sb).
  - LN mean via scalar-engine accum_out, sum(x^2) via DVE stt accum (LN1)
    or rides the residual-add accum (LN2); hn in bf16 directly; LN2 lagged
    4 tiles behind proj and its tail interleaved with fc1 slab-0 m-groups.
  - attention: scoresT = k^T.T @ q^T per head-half (bf16, PE row groups);
    score matmuls fill PSUM bank-pairs so each Exp covers [128,1024];
    E/v/h1 in fp8e4 with DoubleRow matmuls for qkv/v/attnv (halves the
    instruction count; NB DoubleRow streams at the same 1 col/cycle as
    bf16 on this HW, so it only pays off where per-instruction overhead is
    exposed); softmax denominator via ones-slot in the 68-wide padded
    v-aug blocks (16B stride alignment required by dual-fp8 LDWEIGHTS);
    1/den via vector reciprocal from a partition-0 r tile (the custom-DVE
    reciprocal_approx ISA ops produce garbage on this HW, and ISA ops
    mishandle nonzero partition offsets); attnv halves interleaved between
    score halves so the normalize chain hides under the next half.
  - k feature-major tiles computed one slab ahead with the bias-cast on
    DVE (emitted before the reciprocals so the queue never blocks them);
    q bias-cast on scalar (Identity shares the Exp act table); sqrt table
    pre-warmed during the DMA preamble.
  - fc1 m-outer over [128,1024] 2-slab psum slots was reverted to
    slab-split so slab 0 starts before LN2 of tiles 4..7; fc2 bf16 (fp8
    in the MLP costs ~2e-2 rel err, over budget; attention fp8 costs
    ~5e-4).
  - weights bf16/fp8e4 cast on host with LN scale/shift folded in;
    accumulation and the residual path in fp32.
"""

import numpy as np

P = 128
N = 1024          # tokens per core
C = 768           # embed
H = 12            # heads
D = 64            # head dim
HID = 3072
NT = N // P       # 8 token tiles
CK = C // P       # 6 feature k-tiles
HK = HID // P     # 24
VB = 68           # padded per-head v block (16B-friendly)
VW = H * VB       # 816 = v-aug width
EPS = 1e-5
NSLABS = 2
NSL = N // NSLABS  # 512
HPAIRS = H // 2    # 6


def _emit(tc, io, gelu_mode="hw", mm_dt="bf16"):
    """Emit the whole block into TileContext tc. io: dict name->AP."""
    from contextlib import ExitStack

    from concourse import mybir
    from concourse.masks import make_identity

    nc = tc.nc
    fp32 = mybir.dt.float32
    bf16 = mybir.dt.bfloat16
    f8 = mybir.dt.float8e4
    DR = mybir.MatmulPerfMode.DoubleRow
    AF = mybir.ActivationFunctionType
    ALU = mybir.AluOpType

    with ExitStack() as ctx:
        const = ctx.enter_context(tc.tile_pool(name="const", bufs=1))
        work = ctx.enter_context(tc.tile_pool(name="work", bufs=2))
        xpool = ctx.enter_context(tc.tile_pool(name="xpool", bufs=1))
        h2Tpool = ctx.enter_context(tc.tile_pool(name="h2Tpool", bufs=1))
        wprojp = ctx.enter_context(tc.tile_pool(name="wproj", bufs=1))
        # one [P,1024] fp32 psum tag shared by every matmul phase: no pool
        # boundary between attention / proj / MLP means no PE drain stalls
        psM = ctx.enter_context(tc.tile_pool(name="psM", bufs=3, space="PSUM"))

        # x tiles: paired-tile DMAs, one ring trigger per 2 tiles, spread
        # across FOUR engine queues -- a single trigger costs ~2.1us of the
        # issuing engine's time, so serializing all four on one ring delays
        # the data ~8us
        xq = [xpool.tile([P, 2, C], fp32, tag=f"xq_{u}", name=f"xq_{u}")
              for u in range(NT // 2)]
        x_sb = [xq[t // 2][:, t % 2, :] for t in range(NT)]
        # only SP (sync), Activation (scalar) and gpsimd can initiate DMAs;
        # two rings x two triggers each still beats four serialized triggers
        xqueues = [nc.sync, nc.scalar, nc.sync, nc.scalar]
        for u in range(NT // 2):
            xqueues[u].dma_start(
                out=xq[u],
                in_=io["x"][2 * u * P:(2 * u + 2) * P, :].rearrange(
                    "(i p) c -> p i c", p=P))

        # identity matrices (gpsimd, cheap, needed by transposes)
        ident = const.tile([P, P], bf16, tag="ident", name="ident")
        make_identity(nc, ident)
        ident32 = const.tile([HK, HK], fp32, tag="ident32", name="ident32")
        make_identity(nc, ident32)
        eps_t = const.tile([P, 1], fp32, tag="eps", name="eps")
        nc.vector.memset(eps_t, EPS)
        warm = work.tile([P, 1], fp32, tag="warm", name="warm", bufs=1)
        nc.scalar.activation(out=warm, in_=eps_t, func=AF.Sqrt,
                             bias=eps_t, scale=1.0)

        def row_bcast(nm, width):
            row = const.tile([1, width], fp32, tag=f"r_{nm}", name=f"r_{nm}")
            nc.gpsimd.dma_start(
                out=row, in_=io[nm].rearrange("(a w) -> a w", a=1))
            t = const.tile([P, width], fp32, tag=f"bc_{nm}", name=f"bc_{nm}")
            nc.gpsimd.partition_broadcast(t, row)
            return t

        ones_row = const.tile([1, P], bf16, tag="ones_row", name="ones_row")
        nc.vector.memset(ones_row, 1.0)
        b_v_row = const.tile([1, VW], bf16, tag="bvrow", name="b_v_row")
        nc.gpsimd.dma_start(
            out=b_v_row, in_=io["b_v_aug"].rearrange("(a w) -> a w", a=1))

        # col-bias sources [m, 128] (transposed on PE once tiles land);
        # DMAs issued inside phase 1 AFTER the w_v transfers (V matmuls
        # need w_v first; these only feed idle-PE transposes)
        bqk_src = const.tile([2 * CK, P], fp32, tag="bqk_src", name="bqk_src")
        bfc1_src = const.tile([HK, P], fp32, tag="bfc1_src", name="bfc1_src")
        b_qk_col = const.tile([P, 2 * CK], fp32, tag="bqk_col", name="b_qk_col")
        b_fc1_col = const.tile([P, HK], fp32, tag="bfc1_col", name="b_fc1_col")

        # h2T: one [P, CK, N] tile -> LN2 transposes land with ONE wide copy
        h2T = h2Tpool.tile([P, CK, N], bf16, tag="h2T", name="h2T")

        w_proj_sb = []
        for k in range(CK):
            wt = wprojp.tile([P, C], bf16, tag=f"wp_{k}", name=f"wp_{k}")
            w_proj_sb.append(wt)

        # ---------------- LN helper (scalar-engine stats) ----------------
        def ln_stats_hn(src, sum_in=None):
            """mean/var stats split across Scalar/DVE; hn on gpsimd."""
            if sum_in is None:
                scr = work.tile([P, C], f8, tag="scr", name="scr", bufs=1)
                mean = work.tile([P, 1], fp32, tag="s1", name="s1")
                nc.scalar.activation(out=scr, in_=src, func=AF.Copy,
                                     scale=1.0 / C, accum_out=mean)
                # sum of squares on DVE (stt accum), scaled later
                scr2 = work.tile([P, C], fp32, tag="scrv", name="scrv",
                                 bufs=1)
                s2r = work.tile([P, 1], fp32, tag="s2", name="s2")
                nc.vector.scalar_tensor_tensor(
                    out=scr2, in0=src, scalar=1.0, in1=src,
                    op0=ALU.mult, op1=ALU.mult, accum_out=s2r)
                s2scale = 1.0 / C
            else:
                mean = work.tile([P, 1], fp32, tag="s1", name="s1")
                nc.vector.tensor_scalar(out=mean, in0=sum_in,
                                        scalar1=1.0 / C, scalar2=None,
                                        op0=ALU.mult)
                scr2 = work.tile([P, C], f8, tag="scr", name="scr2", bufs=1)
                s2r = work.tile([P, 1], fp32, tag="s2", name="s2")
                nc.scalar.activation(out=scr2, in_=src, func=AF.Square,
                                     scale=(1.0 / C) ** 0.5, accum_out=s2r)
                s2scale = 1.0
            m2 = work.tile([P, 1], fp32, tag="m2", name="m2")
            nc.vector.tensor_mul(out=m2, in0=mean, in1=mean)
            vv = work.tile([P, 1], fp32, tag="vv", name="vv")
            nc.vector.scalar_tensor_tensor(
                out=vv, in0=s2r, scalar=s2scale, in1=m2,
                op0=ALU.mult, op1=ALU.subtract)
            std = work.tile([P, 1], fp32, tag="std", name="std")
            nc.scalar.activation(out=std, in_=vv, func=AF.Sqrt,
                                 bias=eps_t, scale=1.0)
            istd = work.tile([P, 1], fp32, tag="istd", name="istd")
            nc.vector.reciprocal(out=istd, in_=std)
            hn = work.tile([P, C], bf16, tag="hn", name="hn")
            nc.vector.tensor_scalar(out=hn, in0=src,
                                    scalar1=mean, scalar2=istd,
                                    op0=ALU.subtract, op1=ALU.mult)
            return hn

        def transpose_tile(hn, psTpool, dstT, t, eng):
            """6 transposes into one psum bank + one wide strided copy."""
            pt6 = psTpool.tile([P, C], bf16, tag="tr", name="tr")
            for c in range(CK):
                nc.tensor.transpose(pt6[:, c * P:(c + 1) * P],
                                    hn[:, c * P:(c + 1) * P], ident)
            half = CK // 2
            src = pt6.rearrange("p (c q) -> p c q", c=CK)
            dst_lo = dstT[:, :half, t * P:(t + 1) * P]
            dst_hi = dstT[:, half:, t * P:(t + 1) * P]
            if eng == "s":
                nc.scalar.copy(out=dst_lo, in_=src[:, :half, :])
                nc.vector.tensor_copy(out=dst_hi, in_=src[:, half:, :])
            else:
                nc.vector.tensor_copy(out=dst_lo, in_=src[:, :half, :])
                nc.scalar.copy(out=dst_hi, in_=src[:, half:, :])

        with tc.tile_pool(name="aopool", bufs=1) as aopool:
            attn_oT = [aopool.tile([P, N], bf16, tag=f"aoT_{c}",
                                   name=f"aoT_{c}") for c in range(CK)]

            with tc.tile_pool(name="wqk", bufs=1) as wqk_pool, \
                 tc.tile_pool(name="h1Tpool", bufs=1) as h1Tpool, \
                 tc.tile_pool(name="vpool", bufs=1) as vpool:

                w_qk_sb = []
                for j in range(CK // 2):
                    wt = wqk_pool.tile([P, 2, 2 * C], f8, tag=f"wqk_{j}",
                                       name=f"wqk_{j}")
                    w_qk_sb.append(wt)
                h1T = h1Tpool.tile([P, CK, N], f8, tag="h1T", name="h1T")
                v_sb = vpool.tile([P, NT, VW], f8, tag="v_sb", name="v_sb")

                # ---------- phase 1: LN1 + V ----------
                with tc.tile_pool(name="wv", bufs=1) as wv_pool, \
                     tc.tile_pool(name="psT", bufs=2, space="PSUM") as psT:
                    w_v_sb = []
                    for j in range(CK // 2):
                        wt = wv_pool.tile([P, 2, VW], f8, tag=f"wv_{j}",
                                          name=f"wv_{j}")
                        nc.gpsimd.dma_start(
                            out=wt,
                            in_=io["w_v_aug"][2 * j * P:(2 * j + 2) * P,
                                              :].rearrange(
                                "(i p) c -> p i c", p=P))
                        w_v_sb.append(wt)
                    nc.gpsimd.dma_start(
                        out=bqk_src,
                        in_=io["b_qk"].rearrange("(o p) -> o p", p=P))
                    nc.gpsimd.dma_start(
                        out=bfc1_src,
                        in_=io["b_fc1"].rearrange("(o p) -> o p", p=P))

                    # col-bias transposes (PE idles here anyway)
                    ptb = psT.tile([P, HK], fp32, tag="tr", name="ptb")
                    nc.tensor.transpose(ptb[:, :2 * CK], bqk_src,
                                        ident32[:2 * CK, :2 * CK])
                    nc.vector.tensor_copy(out=b_qk_col, in_=ptb[:, :2 * CK])
                    ptb2 = psT.tile([P, HK], fp32, tag="tr", name="ptb2")
                    nc.tensor.transpose(ptb2, bfc1_src, ident32)
                    nc.vector.tensor_copy(out=b_fc1_col, in_=ptb2)

                    for t in range(NT):
                        hn = ln_stats_hn(x_sb[t])
                        transpose_tile(hn, psT, h1T, t,
                                       "s" if t % 2 else "v")
                        ps = psM.tile([P, 2 * NSL], fp32, tag="mm",
                                      name="mm")
                        for off, w in ((0, NSL), (NSL, VW - NSL)):
                            nc.tensor.matmul(
                                ps[:, off:off + w], ones_row,
                                b_v_row[:, off:off + w],
                                start=True, stop=False)
                            for j in range(CK // 2):
                                nc.tensor.matmul(
                                    ps[:, off:off + w],
                                    h1T[:, 2 * j:2 * j + 2,
                                        t * P:(t + 1) * P],
                                    w_v_sb[j][:, :, off:off + w],
                                    start=False, stop=(j == CK // 2 - 1),
                                    perf_mode=DR)
                        nc.scalar.copy(out=v_sb[:, t, :VW // 2],
                                       in_=ps[:, :VW // 2])
                        nc.vector.tensor_copy(out=v_sb[:, t, VW // 2:VW],
                                              in_=ps[:, VW // 2:VW])
                        if 3 <= t < 3 + CK // 2:
                            # wqk transfers deferred so x tiles keep full
                            # DMA bandwidth: pace the gpsimd queue with a
                            # tiny dep on this tile's v output first
                            j = t - 3
                            pace = work.tile([1, 1], f8, tag="pace",
                                             name="pace")
                            nc.gpsimd.tensor_copy(
                                out=pace, in_=v_sb[0:1, t, 0:1])
                            nc.gpsimd.dma_start(
                                out=w_qk_sb[j],
                                in_=io["w_qk"][2 * j * P:(2 * j + 2) * P,
                                               :].rearrange(
                                    "(i p) c -> p i c", p=P))

                # ---------- phase 2+3: attention ----------
                b_proj_bc = row_bcast("b_proj", C)
                for k in range(CK):
                    nc.sync.dma_start(
                        out=w_proj_sb[k],
                        in_=io["w_proj"][k * P:(k + 1) * P, :])
                with tc.tile_pool(name="kqpool", bufs=2) as kqpool, \
                     tc.tile_pool(name="epool", bufs=3) as epool, \
                     tc.tile_pool(name="rpool", bufs=4) as rpool, \
                     tc.tile_pool(name="psV", bufs=2, space="PSUM") as psV:

                    def emit_k(hp):
                        ps = psM.tile([P, N], fp32, tag="mm", name="mm")
                        for ns in range(NSLABS):
                            sl = slice(ns * NSL, (ns + 1) * NSL)
                            for j in range(CK // 2):
                                nc.tensor.matmul(
                                    ps[:, sl],
                                    w_qk_sb[j][:, :, C + hp * P:
                                               C + (hp + 1) * P],
                                    h1T[:, 2 * j:2 * j + 2, sl],
                                    start=(j == 0), stop=(j == CK // 2 - 1),
                                    perf_mode=DR)
                        kt = kqpool.tile([P, N], bf16, tag="ksb", name="ksb")
                        nc.vector.tensor_scalar(
                            out=kt, in0=ps,
                            scalar1=b_qk_col[:, CK + hp:CK + hp + 1],
                            scalar2=None, op0=ALU.add)
                        return kt

                    def emit_q(hp, ns):
                        ps = psM.tile([P, N], fp32, tag="mm", name="mm")
                        sl = slice(ns * NSL, (ns + 1) * NSL)
                        for j in range(CK // 2):
                            nc.tensor.matmul(
                                ps[:, :NSL],
                                w_qk_sb[j][:, :, hp * P:(hp + 1) * P],
                                h1T[:, 2 * j:2 * j + 2, sl],
                                start=(j == 0), stop=(j == CK // 2 - 1),
                                perf_mode=DR)
                        qt = kqpool.tile([P, NSL], bf16, tag="qsb",
                                         name="qsb")
                        # bias-cast on DVE: keeps Scalar exclusively on Exp
                        nc.vector.tensor_scalar(
                            out=qt, in0=ps[:, :NSL],
                            scalar1=b_qk_col[:, hp:hp + 1],
                            scalar2=None, op0=ALU.add)
                        return qt

                    def emit_scores_pair(kt, qt, j, E):
                        # both head-halves' K=64 score matmuls issued
                        # back-to-back: lhsT base partitions 0/64 map to
                        # disjoint PE row-groups, so they run CONCURRENTLY
                        pss = [psM.tile([P, N], fp32, tag="mm", name="mm")
                               for _ in range(2)]
                        for i in range(2):
                            mt = 2 * j + i
                            for half in range(2):
                                pr = slice(half * D, (half + 1) * D)
                                nc.tensor.matmul(
                                    pss[half][:, i * NSL:(i + 1) * NSL],
                                    kt[pr, mt * P:(mt + 1) * P],
                                    qt[pr, :],
                                    start=True, stop=True)
                        for half in range(2):
                            e = epool.tile([P, N], f8,
                                           tag=f"E_{half}_{j}",
                                           name=f"E_{half}_{j}")
                            nc.scalar.activation(out=e, in_=pss[half],
                                                 func=AF.Exp,
                                                 scale=0.125)
                            E[(half, j)] = e

                    def emit_attnv_half(hp, ns, E, half):
                        nsl = slice(ns * NSL, (ns + 1) * NSL)
                        if True:
                            h = 2 * hp + half
                            ps_o = psV.tile([P, NSL], fp32, tag="vo",
                                            name="vo")[:D + 1, :]
                            for u in range(NT // 2):
                                e = E[(half, u)]
                                nc.tensor.matmul(
                                    ps_o,
                                    v_sb[:, 2 * u:2 * u + 2,
                                         h * VB:h * VB + D + 1],
                                    e.rearrange("p (i n) -> p i n", i=2),
                                    start=(u == 0), stop=(u == NT // 2 - 1),
                                    perf_mode=DR)
                            # fast custom-DVE reciprocal: 5x cheaper than
                            # InstReciprocal -- but it computes garbage for
                            # nonzero partition bases (verified: NaN), so
                            # stage den from partition 64 to 0 first
                            dn = rpool.tile([1, NSL], fp32, tag="dn",
                                            name="dn")
                            nc.vector.tensor_copy(
                                out=dn, in_=ps_o[D:D + 1, :])
                            r = rpool.tile([1, NSL], fp32, tag="r",
                                           name="r")
                            nc.vector.reciprocal_approx_fast(
                                out=r, in_=dn)
                            rb = rpool.tile([D, NSL], fp32, tag="rb",
                                            name="rb")
                            nc.gpsimd.partition_broadcast(rb, r)
                            nc.vector.tensor_mul(
                                out=attn_oT[hp][half * D:(half + 1) * D,
                                                nsl],
                                in0=ps_o[:D, :], in1=rb)

                    prev = None
                    x2q = list(range(NT))  # b_proj pre-add (in-place x)
                    kt_cur = emit_k(0)
                    kt_next = None
                    for hp in range(HPAIRS):
                        for ns in range(NSLABS):
                            qt = emit_q(hp, ns)
                            E = {}
                            emit_scores_pair(kt_cur, qt, 0, E)
                            if prev is not None:
                                emit_attnv_half(*prev, 0)
                            emit_scores_pair(kt_cur, qt, 1, E)
                            if prev is not None:
                                emit_attnv_half(*prev, 1)
                            if ns == 0 and hp + 1 < HPAIRS:
                                kt_next = emit_k(hp + 1)
                            emit_scores_pair(kt_cur, qt, 2, E)
                            emit_scores_pair(kt_cur, qt, 3, E)
                            if x2q:
                                # DVE (has slack): gpsimd's strict FIFO
                                # would stall the partition_broadcasts
                                t = x2q.pop(0)
                                nc.vector.tensor_add(
                                    out=x_sb[t], in0=x_sb[t],
                                    in1=b_proj_bc)
                            prev = (hp, ns, E)
                        kt_cur = kt_next
                    emit_attnv_half(*prev, 0)
                    emit_attnv_half(*prev, 1)

            # ---------- phase 4+5+6: proj + LN2 (lagged) + MLP ----------
            b_fc2_bc = row_bcast("b_fc2", C)
            with tc.tile_pool(name="wfc1", bufs=1) as w1pool, \
                 tc.tile_pool(name="wfc2", bufs=1) as w2pool, \
                 tc.tile_pool(name="psT2", bufs=2, space="PSUM") as psT2:
                w1 = []
                for k in range(CK):
                    wt = w1pool.tile([P, HID], bf16, tag=f"wfc1_{k}",
                                     name=f"wfc1_{k}")
                    nc.sync.dma_start(
                        out=wt, in_=io["w_fc1"][k * P:(k + 1) * P, :])
                    w1.append(wt)
                w2g = []
                for g in range(CK):
                    wt = w2pool.tile([P, HK // CK, C], bf16, tag=f"wfc2_{g}",
                                     name=f"wfc2_{g}")
                    nc.sync.dma_start(
                        out=wt,
                        in_=io["w_fc2"][g * 512:(g + 1) * 512, :].rearrange(
                            "(o p) c -> p o c", p=P))
                    w2g.append(wt)

                def emit_proj(t):
                    ps = psM.tile([P, 2 * NSL], fp32, tag="mm", name="mm")
                    for off, w in ((0, NSL), (NSL, C - NSL)):
                        for k in range(CK):
                            nc.tensor.matmul(
                                ps[:, off:off + w],
                                attn_oT[k][:, t * P:(t + 1) * P],
                                w_proj_sb[k][:, off:off + w],
                                start=(k == 0), stop=(k == CK - 1))
                    s1raw = work.tile([P, 1], fp32, tag="s1r", name="s1r")
                    nc.vector.scalar_tensor_tensor(
                        out=x_sb[t], in0=ps[:, :C], scalar=1.0,
                        in1=x_sb[t], op0=ALU.mult, op1=ALU.add,
                        accum_out=s1raw)
                    return s1raw

                def emit_ln2(t, s1raw):
                    hn = ln_stats_hn(x_sb[t], sum_in=s1raw)
                    transpose_tile(hn, psT2, h2T, t, "s" if t % 2 else "v")

                LAG = 2
                s1s = {}
                for t in range(NT):
                    s1s[t] = emit_proj(t)
                    if t >= LAG:
                        emit_ln2(t - LAG, s1s.pop(t - LAG))

                gelu_f = AF.Gelu if gelu_mode == "hw" else AF.Identity
                with tc.tile_pool(name="gpool", bufs=1) as gpool, \
                     tc.tile_pool(name="opool", bufs=2) as opool:
                    gT = gpool.tile([P, HK, N], bf16, tag="gT", name="gT")
                    x2q = list(range(NT))  # b_fc2 pre-add (in-place x)

                    def emit_fc1(ns, ms):
                        sl = slice(ns * NSL, (ns + 1) * NSL)
                        for m in ms:
                            ps = psM.tile([P, 2 * NSL], fp32, tag="mm",
                                          name="mm")
                            for k in range(CK):
                                nc.tensor.matmul(
                                    ps[:, :NSL],
                                    w1[k][:, m * P:(m + 1) * P],
                                    h2T[:, k, sl],
                                    start=(k == 0), stop=(k == CK - 1))
                            nc.scalar.activation(
                                out=gT[:, m, sl], in_=ps[:, :NSL],
                                func=gelu_f,
                                bias=b_fc1_col[:, m:m + 1], scale=1.0)

                    # LN2 tail FIRST (all Sqrts contiguous -> one act-table
                    # load), then all fc1 gelus (one gelu table load); the
                    # baseline interleave thrashed 5 table loads (~13us of
                    # Scalar time in the fc1-slab-0 window)
                    for t in range(NT - LAG, NT):
                        emit_ln2(t, s1s.pop(t))
                    for i in range(4):
                        emit_fc1(0, range(i * 6, (i + 1) * 6))
                        if x2q:
                            tt = x2q.pop(0)
                            nc.gpsimd.tensor_add(
                                out=x_sb[tt], in0=x_sb[tt], in1=b_fc2_bc)
                    for i in range(4):
                        emit_fc1(1, range(i * 6, (i + 1) * 6))
                        if x2q:
                            tt = x2q.pop(0)
                            nc.gpsimd.tensor_add(
                                out=x_sb[tt], in0=x_sb[tt], in1=b_fc2_bc)
                    while x2q:
                        tt = x2q.pop(0)
                        nc.vector.tensor_add(
                            out=x_sb[tt], in0=x_sb[tt], in1=b_fc2_bc)

                    for t in range(NT):
                        ps = psM.tile([P, 2 * NSL], fp32, tag="mm",
                                      name="mm")
                        ot = opool.tile([P, C], fp32, tag="ot", name="ot")
                        for off, w in ((0, NSL), (NSL, C - NSL)):
                            for k in range(HK):
                                nc.tensor.matmul(
                                    ps[:, off:off + w],
                                    gT[:, k, t * P:(t + 1) * P],
                                    w2g[k // 4][:, k % 4, off:off + w],
                                    start=(k == 0), stop=(k == HK - 1))
                            nc.vector.tensor_add(
                                out=ot[:, off:off + w],
                                in0=ps[:, off:off + w],
                                in1=x_sb[t][:, off:off + w])
                            nc.sync.dma_start(
                                out=io["out"][t * P:(t + 1) * P,
                                              off:off + w],
                                in_=ot[:, off:off + w])


def build_program(gelu_mode="hw", mm_dt="bf16"):
    import concourse.tile as tile
    from concourse import bacc, mybir

    fp32 = mybir.dt.float32
    bf16 = mybir.dt.bfloat16
    nc = bacc.Bacc("TRN2", target_bir_lowering=False, debug=False,
                   num_devices=8)

    f8 = mybir.dt.float8e4
    shapes = {
        "x": ([N, C], fp32),
        "w_qk": ([C, 2 * C], f8), "b_qk": ([2 * C], fp32),
        "w_v_aug": ([C, VW], f8), "b_v_aug": ([VW], fp32),
        "w_proj": ([C, C], bf16), "b_proj": ([C], fp32),
        "w_fc1": ([C, HID], bf16), "b_fc1": ([HID], fp32),
        "w_fc2": ([HID, C], bf16), "b_fc2": ([C], fp32),
    }
    io = {}
    for name, (shp, dt) in shapes.items():
        io[name] = nc.dram_tensor(name, shp, dt, kind="ExternalInput").ap()
    io["out"] = nc.dram_tensor("out", [N, C], fp32, kind="ExternalOutput").ap()

    with tile.TileContext(nc) as tc:
        _emit(tc, io, gelu_mode=gelu_mode, mm_dt=mm_dt)
    nc.compile()
    return nc


def host_prep(inputs, mm_dt="bf16"):
    """Fold LN1/LN2 scale+shift into w_qk/w_v/w_fc1; build v-aug layout;
    cast weights to bf16."""
    import ml_dtypes
    f32 = np.float32
    bf = ml_dtypes.bfloat16

    x = np.asarray(inputs["x"], f32)
    w_qkv = np.asarray(inputs["w_qkv"], f32)
    b_qkv = np.asarray(inputs["b_qkv"], f32)
    ln1_w = np.asarray(inputs["ln1_w"], f32)
    ln1_b = np.asarray(inputs["ln1_b"], f32)
    ln2_w = np.asarray(inputs["ln2_w"], f32)
    ln2_b = np.asarray(inputs["ln2_b"], f32)

    w_q = w_qkv[:, 0:C]
    w_k = w_qkv[:, C:2 * C]
    w_v = w_qkv[:, 2 * C:3 * C]
    b_q = b_qkv[0:C]
    b_k = b_qkv[C:2 * C]
    b_v = b_qkv[2 * C:3 * C]

    # fold LN1: h = hn*ln1_w + ln1_b  =>  W' = ln1_w[:,None]*W, b' = b + W.T@ln1_b
    w_qk = np.concatenate([w_q, w_k], axis=1)          # [C, 2C]
    w_qk_f = ln1_w[:, None] * w_qk
    b_qk_f = np.concatenate([b_q, b_k]) + w_qk.T @ ln1_b

    w_v_f = ln1_w[:, None] * w_v
    b_v_f = b_v + w_v.T @ ln1_b
    w_v_aug = np.zeros((C, VW), f32)
    b_v_aug = np.zeros((VW,), f32)
    for h in range(H):
        w_v_aug[:, h * VB:h * VB + D] = w_v_f[:, h * D:(h + 1) * D]
        b_v_aug[h * VB:h * VB + D] = b_v_f[h * D:(h + 1) * D]
        b_v_aug[h * VB + D] = 1.0

    w_fc1 = np.asarray(inputs["w_fc1"], f32)
    b_fc1 = np.asarray(inputs["b_fc1"], f32)
    w_fc1_f = ln2_w[:, None] * w_fc1
    b_fc1_f = b_fc1 + w_fc1.T @ ln2_b

    def f8(a):
        # TRN e4m3 matches OCP e4m3fn bit-for-bit for |x| <= 240
        return np.ascontiguousarray(
            np.clip(a, -240, 240)).astype(ml_dtypes.float8_e4m3)

    common = {
        "w_qk": f8(w_qk_f),
        "b_qk": np.ascontiguousarray(b_qk_f, f32),
        "w_v_aug": f8(w_v_aug),
        "b_v_aug": b_v_aug,
        "w_proj": np.ascontiguousarray(
            np.asarray(inputs["w_proj"], f32)).astype(bf),
        "b_proj": np.ascontiguousarray(np.asarray(inputs["b_proj"], f32)),
        "w_fc1": np.ascontiguousarray(w_fc1_f).astype(bf),
        "b_fc1": np.ascontiguousarray(b_fc1_f, f32),
        "w_fc2": np.ascontiguousarray(np.asarray(inputs["w_fc2"], f32)).astype(bf),
        "b_fc2": np.ascontiguousarray(np.asarray(inputs["b_fc2"], f32)),
    }
    in_maps = []
    for i in range(x.shape[0]):
        m = dict(common)
        m["x"] = np.ascontiguousarray(x[i])
        in_maps.append(m)
    return in_maps


_CACHE = {}


def kernel(**inputs):
    from concourse.bass_utils import run_bass_kernel_spmd

    if "nc" not in _CACHE:
        _CACHE["nc"] = build_program(gelu_mode="hw")
    nc = _CACHE["nc"]
    in_maps = host_prep(inputs)
    res = run_bass_kernel_spmd(nc, in_maps, list(range(8)))
    out = np.stack([r["out"] for r in res.results], axis=0)
    return out.astype(np.float32)



# revision 17
# speedup vs baseline: 1.0005x; 1.0005x over previous
"""Transformer block (LN->MHA->residual->LN->MLP->residual) on 8 trn2 cores.

Data-parallel over batch: each of the 8 NeuronCores processes one [1024, 768]
batch element with the full weight set.  No collectives.

Tuned from the 467us baseline via trace analysis (final ~335-355us):
  - token-major for LN stats/residuals/output; feature-major (PE transposes)
    for matmul operands; one [P,1024] fp32 PSUM tag shared by every matmul
    phase so no pool boundary (= no PE drain stall) between attention, proj
    and MLP; attnv gets a private 2-buf PSUM pool.
  - x loaded as paired-tile DMAs on the sync queue ahead of everything
    (single-ring DMA triggers serialize ~2.6us/tile); w_qk transfers paced
    behind x via a gpsimd data dependency; w_fc1/w_fc2 loaded during proj.
  - column biases (b_qk, b_fc1) loaded as [m,128] rows and PE-transposed
    (a 4-byte-element gather DMA costs >20us); row biases applied via K=1
    ones-row matmul PSUM preload (v) or pre-added into the residual on
    idle DVE cycles (b_proj, b_fc2, in-place in x_sb).
  - LN mean via scalar-engine accum_out, sum(x^2) via DVE stt accum (LN1)
    or rides the residual-add accum (LN2); hn in bf16 directly; LN2 lagged
    4 tiles behind proj and its tail interleaved with fc1 slab-0 m-groups.
  - attention: scoresT = k^T.T @ q^T per head-half (bf16, PE row groups);
    score matmuls fill PSUM bank-pairs so each Exp covers [128,1024];
    E/v/h1 in fp8e4 with DoubleRow matmuls for qkv/v/attnv (halves the
    instruction count; NB DoubleRow streams at the same 1 col/cycle as
    bf16 on this HW, so it only pays off where per-instruction overhead is
    exposed); softmax denominator via ones-slot in the 68-wide padded
    v-aug blocks (16B stride alignment required by dual-fp8 LDWEIGHTS);
    1/den via vector reciprocal from a partition-0 r tile (the custom-DVE
    reciprocal_approx ISA ops produce garbage on this HW, and ISA ops
    mishandle nonzero partition offsets); attnv halves interleaved between
    score halves so the normalize chain hides under the next half.
  - k feature-major tiles computed one slab ahead with the bias-cast on
    DVE (emitted before the reciprocals so the queue never blocks them);
    q bias-cast on scalar (Identity shares the Exp act table); sqrt table
    pre-warmed during the DMA preamble.
  - fc1 m-outer over [128,1024] 2-slab psum slots was reverted to
    slab-split so slab 0 starts before LN2 of tiles 4..7; fc2 bf16 (fp8
    in the MLP costs ~2e-2 rel err, over budget; attention fp8 costs
    ~5e-4).
  - weights bf16/fp8e4 cast on host with LN scale/shift folded in;
    accumulation and the residual path in fp32.
"""

import numpy as np

P = 128
N = 1024          # tokens per core
C = 768           # embed
H = 12            # heads
D = 64            # head dim
HID = 3072
NT = N // P       # 8 token tiles
CK = C // P       # 6 feature k-tiles
HK = HID // P     # 24
VB = 68           # padded per-head v block (16B-friendly)
VW = H * VB       # 816 = v-aug width
EPS = 1e-5
NSLABS = 2
NSL = N // NSLABS  # 512
HPAIRS = H // 2    # 6


def _emit(tc, io, gelu_mode="hw", mm_dt="bf16"):
    """Emit the whole block into TileContext tc. io: dict name->AP."""
    from contextlib import ExitStack

    from concourse import mybir
    from concourse.masks import make_identity

    nc = tc.nc
    fp32 = mybir.dt.float32
    bf16 = mybir.dt.bfloat16
    f8 = mybir.dt.float8e4
    DR = mybir.MatmulPerfMode.DoubleRow
    AF = mybir.ActivationFunctionType
    ALU = mybir.AluOpType

    with ExitStack() as ctx:
        const = ctx.enter_context(tc.tile_pool(name="const", bufs=1))
        work = ctx.enter_context(tc.tile_pool(name="work", bufs=2))
        xpool = ctx.enter_context(tc.tile_pool(name="xpool", bufs=1))
        h2Tpool = ctx.enter_context(tc.tile_pool(name="h2Tpool", bufs=1))
        wprojp = ctx.enter_context(tc.tile_pool(name="wproj", bufs=1))
        # one [P,1024] fp32 psum tag shared by every matmul phase: no pool
        # boundary between attention / proj / MLP means no PE drain stalls
        psM = ctx.enter_context(tc.tile_pool(name="psM", bufs=3, space="PSUM"))

        # x tiles: paired-tile DMAs, one ring trigger per 2 tiles, spread
        # across FOUR engine queues -- a single trigger costs ~2.1us of the
        # issuing engine's time, so serializing all four on one ring delays
        # the data ~8us
        xq = [xpool.tile([P, 2, C], fp32, tag=f"xq_{u}", name=f"xq_{u}")
              for u in range(NT // 2)]
        x_sb = [xq[t // 2][:, t % 2, :] for t in range(NT)]
        # only SP (sync), Activation (scalar) and gpsimd can initiate DMAs;
        # two rings x two triggers each still beats four serialized triggers
        xqueues = [nc.sync, nc.scalar, nc.sync, nc.scalar]
        for u in range(NT // 2):
            xqueues[u].dma_start(
                out=xq[u],
                in_=io["x"][2 * u * P:(2 * u + 2) * P, :].rearrange(
                    "(i p) c -> p i c", p=P))

        # identity matrices (gpsimd, cheap, needed by transposes)
        ident = const.tile([P, P], bf16, tag="ident", name="ident")
        make_identity(nc, ident)
        ident32 = const.tile([HK, HK], fp32, tag="ident32", name="ident32")
        make_identity(nc, ident32)
        eps_t = const.tile([P, 1], fp32, tag="eps", name="eps")
        nc.vector.memset(eps_t, EPS)
        warm = work.tile([P, 1], fp32, tag="warm", name="warm", bufs=1)
        nc.scalar.activation(out=warm, in_=eps_t, func=AF.Sqrt,
                             bias=eps_t, scale=1.0)

        def row_bcast(nm, width):
            row = const.tile([1, width], fp32, tag=f"r_{nm}", name=f"r_{nm}")
            nc.gpsimd.dma_start(
                out=row, in_=io[nm].rearrange("(a w) -> a w", a=1))
            t = const.tile([P, width], fp32, tag=f"bc_{nm}", name=f"bc_{nm}")
            nc.gpsimd.partition_broadcast(t, row)
            return t

        ones_row = const.tile([1, P], bf16, tag="ones_row", name="ones_row")
        nc.vector.memset(ones_row, 1.0)
        b_v_row = const.tile([1, VW], bf16, tag="bvrow", name="b_v_row")
        nc.gpsimd.dma_start(
            out=b_v_row, in_=io["b_v_aug"].rearrange("(a w) -> a w", a=1))

        # col-bias sources [m, 128] (transposed on PE once tiles land);
        # DMAs issued inside phase 1 AFTER the w_v transfers (V matmuls
        # need w_v first; these only feed idle-PE transposes)
        bqk_src = const.tile([2 * CK, P], fp32, tag="bqk_src", name="bqk_src")
        bfc1_src = const.tile([HK, P], fp32, tag="bfc1_src", name="bfc1_src")
        b_qk_col = const.tile([P, 2 * CK], fp32, tag="bqk_col", name="b_qk_col")
        b_fc1_col = const.tile([P, HK], fp32, tag="bfc1_col", name="b_fc1_col")

        # h2T: one [P, CK, N] tile -> LN2 transposes land with ONE wide copy
        h2T = h2Tpool.tile([P, CK, N], bf16, tag="h2T", name="h2T")

        w_proj_sb = []
        for k in range(CK):
            wt = wprojp.tile([P, C], bf16, tag=f"wp_{k}", name=f"wp_{k}")
            w_proj_sb.append(wt)

        # ---------------- LN helper (scalar-engine stats) ----------------
        def ln_stats_hn(src, sum_in=None):
            """mean/var stats split across Scalar/DVE; hn on gpsimd."""
            if sum_in is None:
                scr = work.tile([P, C], f8, tag="scr", name="scr", bufs=1)
                mean = work.tile([P, 1], fp32, tag="s1", name="s1")
                nc.scalar.activation(out=scr, in_=src, func=AF.Copy,
                                     scale=1.0 / C, accum_out=mean)
                # sum of squares on DVE (stt accum), scaled later
                scr2 = work.tile([P, C], fp32, tag="scrv", name="scrv",
                                 bufs=1)
                s2r = work.tile([P, 1], fp32, tag="s2", name="s2")
                nc.vector.scalar_tensor_tensor(
                    out=scr2, in0=src, scalar=1.0, in1=src,
                    op0=ALU.mult, op1=ALU.mult, accum_out=s2r)
                s2scale = 1.0 / C
            else:
                mean = work.tile([P, 1], fp32, tag="s1", name="s1")
                nc.vector.tensor_scalar(out=mean, in0=sum_in,
                                        scalar1=1.0 / C, scalar2=None,
                                        op0=ALU.mult)
                scr2 = work.tile([P, C], f8, tag="scr", name="scr2", bufs=1)
                s2r = work.tile([P, 1], fp32, tag="s2", name="s2")
                nc.scalar.activation(out=scr2, in_=src, func=AF.Square,
                                     scale=(1.0 / C) ** 0.5, accum_out=s2r)
                s2scale = 1.0
            m2 = work.tile([P, 1], fp32, tag="m2", name="m2")
            nc.vector.tensor_mul(out=m2, in0=mean, in1=mean)
            vv = work.tile([P, 1], fp32, tag="vv", name="vv")
            nc.vector.scalar_tensor_tensor(
                out=vv, in0=s2r, scalar=s2scale, in1=m2,
                op0=ALU.mult, op1=ALU.subtract)
            std = work.tile([P, 1], fp32, tag="std", name="std")
            nc.scalar.activation(out=std, in_=vv, func=AF.Sqrt,
                                 bias=eps_t, scale=1.0)
            istd = work.tile([P, 1], fp32, tag="istd", name="istd")
            nc.vector.reciprocal(out=istd, in_=std)
            hn = work.tile([P, C], bf16, tag="hn", name="hn")
            nc.vector.tensor_scalar(out=hn, in0=src,
                                    scalar1=mean, scalar2=istd,
                                    op0=ALU.subtract, op1=ALU.mult)
            return hn

        def transpose_tile(hn, psTpool, dstT, t, eng):
            """6 transposes into one psum bank + one wide strided copy."""
            pt6 = psTpool.tile([P, C], bf16, tag="tr", name="tr")
            for c in range(CK):
                nc.tensor.transpose(pt6[:, c * P:(c + 1) * P],
                                    hn[:, c * P:(c + 1) * P], ident)
            half = CK // 2
            src = pt6.rearrange("p (c q) -> p c q", c=CK)
            dst_lo = dstT[:, :half, t * P:(t + 1) * P]
            dst_hi = dstT[:, half:, t * P:(t + 1) * P]
            if eng == "s":
                nc.scalar.copy(out=dst_lo, in_=src[:, :half, :])
                nc.vector.tensor_copy(out=dst_hi, in_=src[:, half:, :])
            else:
                nc.vector.tensor_copy(out=dst_lo, in_=src[:, :half, :])
                nc.scalar.copy(out=dst_hi, in_=src[:, half:, :])

        with tc.tile_pool(name="aopool", bufs=1) as aopool:
            attn_oT = [aopool.tile([P, N], bf16, tag=f"aoT_{c}",
                                   name=f"aoT_{c}") for c in range(CK)]

            with tc.tile_pool(name="wqk", bufs=1) as wqk_pool, \
                 tc.tile_pool(name="h1Tpool", bufs=1) as h1Tpool, \
                 tc.tile_pool(name="vpool", bufs=1) as vpool:

                w_qk_sb = []
                for j in range(CK // 2):
                    wt = wqk_pool.tile([P, 2, 2 * C], f8, tag=f"wqk_{j}",
                                       name=f"wqk_{j}")
                    w_qk_sb.append(wt)
                h1T = h1Tpool.tile([P, CK, N], f8, tag="h1T", name="h1T")
                v_sb = vpool.tile([P, NT, VW], f8, tag="v_sb", name="v_sb")

                # ---------- phase 1: LN1 + V ----------
                with tc.tile_pool(name="wv", bufs=1) as wv_pool, \
                     tc.tile_pool(name="psT", bufs=2, space="PSUM") as psT:
                    w_v_sb = []
                    for j in range(CK // 2):
                        wt = wv_pool.tile([P, 2, VW], f8, tag=f"wv_{j}",
                                          name=f"wv_{j}")
                        nc.gpsimd.dma_start(
                            out=wt,
                            in_=io["w_v_aug"][2 * j * P:(2 * j + 2) * P,
                                              :].rearrange(
                                "(i p) c -> p i c", p=P))
                        w_v_sb.append(wt)
                    nc.gpsimd.dma_start(
                        out=bqk_src,
                        in_=io["b_qk"].rearrange("(o p) -> o p", p=P))
                    nc.gpsimd.dma_start(
                        out=bfc1_src,
                        in_=io["b_fc1"].rearrange("(o p) -> o p", p=P))

                    # col-bias transposes (PE idles here anyway)
                    ptb = psT.tile([P, HK], fp32, tag="tr", name="ptb")
                    nc.tensor.transpose(ptb[:, :2 * CK], bqk_src,
                                        ident32[:2 * CK, :2 * CK])
                    nc.vector.tensor_copy(out=b_qk_col, in_=ptb[:, :2 * CK])
                    ptb2 = psT.tile([P, HK], fp32, tag="tr", name="ptb2")
                    nc.tensor.transpose(ptb2, bfc1_src, ident32)
                    nc.vector.tensor_copy(out=b_fc1_col, in_=ptb2)

                    for t in range(NT):
                        hn = ln_stats_hn(x_sb[t])
                        transpose_tile(hn, psT, h1T, t,
                                       "s" if t % 2 else "v")
                        ps = psM.tile([P, 2 * NSL], fp32, tag="mm",
                                      name="mm")
                        for off, w in ((0, NSL), (NSL, VW - NSL)):
                            nc.tensor.matmul(
                                ps[:, off:off + w], ones_row,
                                b_v_row[:, off:off + w],
                                start=True, stop=False)
                            for j in range(CK // 2):
                                nc.tensor.matmul(
                                    ps[:, off:off + w],
                                    h1T[:, 2 * j:2 * j + 2,
                                        t * P:(t + 1) * P],
                                    w_v_sb[j][:, :, off:off + w],
                                    start=False, stop=(j == CK // 2 - 1),
                                    perf_mode=DR)
                        nc.scalar.copy(out=v_sb[:, t, :VW // 2],
                                       in_=ps[:, :VW // 2])
                        nc.vector.tensor_copy(out=v_sb[:, t, VW // 2:VW],
                                              in_=ps[:, VW // 2:VW])
                        if 3 <= t < 3 + CK // 2:
                            # wqk transfers deferred so x tiles keep full
                            # DMA bandwidth: pace the gpsimd queue with a
                            # tiny dep on this tile's v output first
                            j = t - 3
                            pace = work.tile([1, 1], f8, tag="pace",
                                             name="pace")
                            nc.gpsimd.tensor_copy(
                                out=pace, in_=v_sb[0:1, t, 0:1])
                            nc.gpsimd.dma_start(
                                out=w_qk_sb[j],
                                in_=io["w_qk"][2 * j * P:(2 * j + 2) * P,
                                               :].rearrange(
                                    "(i p) c -> p i c", p=P))

                # ---------- phase 2+3: attention ----------
                b_proj_bc = row_bcast("b_proj", C)
                for k in range(CK):
                    nc.sync.dma_start(
                        out=w_proj_sb[k],
                        in_=io["w_proj"][k * P:(k + 1) * P, :])
                with tc.tile_pool(name="kqpool", bufs=2) as kqpool, \
                     tc.tile_pool(name="epool", bufs=3) as epool, \
                     tc.tile_pool(name="rpool", bufs=4) as rpool, \
                     tc.tile_pool(name="psV", bufs=2, space="PSUM") as psV:

                    def emit_k(hp):
                        ps = psM.tile([P, N], fp32, tag="mm", name="mm")
                        for ns in range(NSLABS):
                            sl = slice(ns * NSL, (ns + 1) * NSL)
                            for j in range(CK // 2):
                                nc.tensor.matmul(
                                    ps[:, sl],
                                    w_qk_sb[j][:, :, C + hp * P:
                                               C + (hp + 1) * P],
                                    h1T[:, 2 * j:2 * j + 2, sl],
                                    start=(j == 0), stop=(j == CK // 2 - 1),
                                    perf_mode=DR)
                        kt = kqpool.tile([P, N], bf16, tag="ksb", name="ksb")
                        nc.vector.tensor_scalar(
                            out=kt, in0=ps,
                            scalar1=b_qk_col[:, CK + hp:CK + hp + 1],
                            scalar2=None, op0=ALU.add)
                        return kt

                    def emit_q(hp, ns):
                        ps = psM.tile([P, N], fp32, tag="mm", name="mm")
                        sl = slice(ns * NSL, (ns + 1) * NSL)
                        for j in range(CK // 2):
                            nc.tensor.matmul(
                                ps[:, :NSL],
                                w_qk_sb[j][:, :, hp * P:(hp + 1) * P],
                                h1T[:, 2 * j:2 * j + 2, sl],
                                start=(j == 0), stop=(j == CK // 2 - 1),
                                perf_mode=DR)
                        qt = kqpool.tile([P, NSL], bf16, tag="qsb",
                                         name="qsb")
                        # bias-cast on DVE: keeps Scalar exclusively on Exp
                        nc.vector.tensor_scalar(
                            out=qt, in0=ps[:, :NSL],
                            scalar1=b_qk_col[:, hp:hp + 1],
                            scalar2=None, op0=ALU.add)
                        return qt

                    def emit_scores_pair(kt, qt, j, E):
                        # both head-halves' K=64 score matmuls issued
                        # back-to-back: lhsT base partitions 0/64 map to
                        # disjoint PE row-groups, so they run CONCURRENTLY
                        pss = [psM.tile([P, N], fp32, tag="mm", name="mm")
                               for _ in range(2)]
                        for i in range(2):
                            mt = 2 * j + i
                            for half in range(2):
                                pr = slice(half * D, (half + 1) * D)
                                nc.tensor.matmul(
                                    pss[half][:, i * NSL:(i + 1) * NSL],
                                    kt[pr, mt * P:(mt + 1) * P],
                                    qt[pr, :],
                                    start=True, stop=True)
                        for half in range(2):
                            e = epool.tile([P, N], f8,
                                           tag=f"E_{half}_{j}",
                                           name=f"E_{half}_{j}")
                            nc.scalar.activation(out=e, in_=pss[half],
                                                 func=AF.Exp,
                                                 scale=0.125)
                            E[(half, j)] = e

                    # normalize chain split into three stages so the DVE's
                    # strict-FIFO queue never blocks on a gpsimd round-trip:
                    # mm+recip first, both bcasts together, mults at the
                    # END of the instance (bcasts are long done by then)
                    def attnv_mm(hp, ns, E, half):
                        h = 2 * hp + half
                        ps_o = psV.tile([P, NSL], fp32, tag="vo",
                                        name="vo")[:D + 1, :]
                        for u in range(NT // 2):
                            e = E[(half, u)]
                            nc.tensor.matmul(
                                ps_o,
                                v_sb[:, 2 * u:2 * u + 2,
                                     h * VB:h * VB + D + 1],
                                e.rearrange("p (i n) -> p i n", i=2),
                                start=(u == 0), stop=(u == NT // 2 - 1),
                                perf_mode=DR)
                        # fast custom-DVE reciprocal: 5x cheaper than
                        # InstReciprocal -- but it computes garbage for
                        # nonzero partition bases (verified: NaN), so
                        # stage den from partition 64 to 0 first
                        dn = rpool.tile([1, NSL], fp32, tag="dn",
                                        name="dn")
                        nc.vector.tensor_copy(
                            out=dn, in_=ps_o[D:D + 1, :])
                        r = rpool.tile([1, NSL], fp32, tag="r",
                                       name="r")
                        nc.vector.reciprocal_approx_fast(out=r, in_=dn)
                        return [hp, ns, half, ps_o, r]

                    def attnv_bcast(a):
                        rb = rpool.tile([D, NSL], fp32, tag="rb",
                                        name="rb")
                        nc.gpsimd.partition_broadcast(rb, a[4])
                        a.append(rb)

                    def attnv_mult(a):
                        hp, ns, half, ps_o, r, rb = a
                        nsl = slice(ns * NSL, (ns + 1) * NSL)
                        nc.vector.tensor_mul(
                            out=attn_oT[hp][half * D:(half + 1) * D, nsl],
                            in0=ps_o[:D, :], in1=rb)

                    prev = None
                    x2q = list(range(NT))  # b_proj pre-add (in-place x)
                    kt_cur = emit_k(0)
                    kt_next = None
                    for hp in range(HPAIRS):
                        for ns in range(NSLABS):
                            qt = emit_q(hp, ns)
                            E = {}
                            emit_scores_pair(kt_cur, qt, 0, E)
                            a0 = attnv_mm(*prev, 0) if prev else None
                            emit_scores_pair(kt_cur, qt, 1, E)
                            if prev is not None:
                                a1 = attnv_mm(*prev, 1)
                                attnv_bcast(a0)
                                attnv_bcast(a1)
                            if ns == 0 and hp + 1 < HPAIRS:
                                kt_next = emit_k(hp + 1)
                            emit_scores_pair(kt_cur, qt, 2, E)
                            emit_scores_pair(kt_cur, qt, 3, E)
                            if prev is not None:
                                attnv_mult(a0)
                                attnv_mult(a1)
                            if x2q:
                                # DVE (has slack): gpsimd's strict FIFO
                                # would stall the partition_broadcasts
                                t = x2q.pop(0)
                                nc.vector.tensor_add(
                                    out=x_sb[t], in0=x_sb[t],
                                    in1=b_proj_bc)
                            prev = (hp, ns, E)
                        kt_cur = kt_next
                    a0 = attnv_mm(*prev, 0)
                    a1 = attnv_mm(*prev, 1)
                    attnv_bcast(a0)
                    attnv_bcast(a1)
                    attnv_mult(a0)
                    attnv_mult(a1)

            # ---------- phase 4+5+6: proj + LN2 (lagged) + MLP ----------
            b_fc2_bc = row_bcast("b_fc2", C)
            with tc.tile_pool(name="wfc1", bufs=1) as w1pool, \
                 tc.tile_pool(name="wfc2", bufs=1) as w2pool, \
                 tc.tile_pool(name="psT2", bufs=2, space="PSUM") as psT2:
                w1 = []
                for k in range(CK):
                    wt = w1pool.tile([P, HID], bf16, tag=f"wfc1_{k}",
                                     name=f"wfc1_{k}")
                    nc.sync.dma_start(
                        out=wt, in_=io["w_fc1"][k * P:(k + 1) * P, :])
                    w1.append(wt)
                w2g = []
                for g in range(CK):
                    wt = w2pool.tile([P, HK // CK, C], bf16, tag=f"wfc2_{g}",
                                     name=f"wfc2_{g}")
                    nc.sync.dma_start(
                        out=wt,
                        in_=io["w_fc2"][g * 512:(g + 1) * 512, :].rearrange(
                            "(o p) c -> p o c", p=P))
                    w2g.append(wt)

                def emit_proj(t):
                    ps = psM.tile([P, 2 * NSL], fp32, tag="mm", name="mm")
                    for off, w in ((0, NSL), (NSL, C - NSL)):
                        for k in range(CK):
                            nc.tensor.matmul(
                                ps[:, off:off + w],
                                attn_oT[k][:, t * P:(t + 1) * P],
                                w_proj_sb[k][:, off:off + w],
                                start=(k == 0), stop=(k == CK - 1))
                    s1raw = work.tile([P, 1], fp32, tag="s1r", name="s1r")
                    nc.vector.scalar_tensor_tensor(
                        out=x_sb[t], in0=ps[:, :C], scalar=1.0,
                        in1=x_sb[t], op0=ALU.mult, op1=ALU.add,
                        accum_out=s1raw)
                    return s1raw

                def emit_ln2(t, s1raw):
                    hn = ln_stats_hn(x_sb[t], sum_in=s1raw)
                    transpose_tile(hn, psT2, h2T, t, "s" if t % 2 else "v")

                LAG = 2
                s1s = {}
                for t in range(NT):
                    s1s[t] = emit_proj(t)
                    if t >= LAG:
                        emit_ln2(t - LAG, s1s.pop(t - LAG))

                gelu_f = AF.Gelu if gelu_mode == "hw" else AF.Identity
                with tc.tile_pool(name="gpool", bufs=1) as gpool, \
                     tc.tile_pool(name="opool", bufs=2) as opool:
                    gT = gpool.tile([P, HK, N], bf16, tag="gT", name="gT")
                    x2q = list(range(NT))  # b_fc2 pre-add (in-place x)

                    def emit_fc1(ns, ms):
                        sl = slice(ns * NSL, (ns + 1) * NSL)
                        for m in ms:
                            ps = psM.tile([P, 2 * NSL], fp32, tag="mm",
                                          name="mm")
                            for k in range(CK):
                                nc.tensor.matmul(
                                    ps[:, :NSL],
                                    w1[k][:, m * P:(m + 1) * P],
                                    h2T[:, k, sl],
                                    start=(k == 0), stop=(k == CK - 1))
                            nc.scalar.activation(
                                out=gT[:, m, sl], in_=ps[:, :NSL],
                                func=gelu_f,
                                bias=b_fc1_col[:, m:m + 1], scale=1.0)

                    # LN2 tail FIRST (all Sqrts contiguous -> one act-table
                    # load), then all fc1 gelus (one gelu table load); the
                    # baseline interleave thrashed 5 table loads (~13us of
                    # Scalar time in the fc1-slab-0 window)
                    for t in range(NT - LAG, NT):
                        emit_ln2(t, s1s.pop(t))
                    for i in range(4):
                        emit_fc1(0, range(i * 6, (i + 1) * 6))
                        if x2q:
                            tt = x2q.pop(0)
                            nc.gpsimd.tensor_add(
                                out=x_sb[tt], in0=x_sb[tt], in1=b_fc2_bc)
                    for i in range(4):
                        emit_fc1(1, range(i * 6, (i + 1) * 6))
                        if x2q:
                            tt = x2q.pop(0)
                            nc.gpsimd.tensor_add(
                                out=x_sb[tt], in0=x_sb[tt], in1=b_fc2_bc)
                    while x2q:
                        tt = x2q.pop(0)
                        nc.vector.tensor_add(
                            out=x_sb[tt], in0=x_sb[tt], in1=b_fc2_bc)

                    for t in range(NT):
                        ps = psM.tile([P, 2 * NSL], fp32, tag="mm",
                                      name="mm")
                        ot = opool.tile([P, C], fp32, tag="ot", name="ot")
                        for off, w in ((0, NSL), (NSL, C - NSL)):
                            for k in range(HK):
                                nc.tensor.matmul(
                                    ps[:, off:off + w],
                                    gT[:, k, t * P:(t + 1) * P],
                                    w2g[k // 4][:, k % 4, off:off + w],
                                    start=(k == 0), stop=(k == HK - 1))
                            nc.vector.tensor_add(
                                out=ot[:, off:off + w],
                                in0=ps[:, off:off + w],
                                in1=x_sb[t][:, off:off + w])
                            nc.sync.dma_start(
                                out=io["out"][t * P:(t + 1) * P,
                                              off:off + w],
                                in_=ot[:, off:off + w])


def build_program(gelu_mode="hw", mm_dt="bf16"):
    import concourse.tile as tile
    from concourse import bacc, mybir

    fp32 = mybir.dt.float32
    bf16 = mybir.dt.bfloat16
    nc = bacc.Bacc("TRN2", target_bir_lowering=False, debug=False,
                   num_devices=8)

    f8 = mybir.dt.float8e4
    shapes = {
        "x": ([N, C], fp32),
        "w_qk": ([C, 2 * C], f8), "b_qk": ([2 * C], fp32),
        "w_v_aug": ([C, VW], f8), "b_v_aug": ([VW], fp32),
        "w_proj": ([C, C], bf16), "b_proj": ([C], fp32),
        "w_fc1": ([C, HID], bf16), "b_fc1": ([HID], fp32),
        "w_fc2": ([HID, C], bf16), "b_fc2": ([C], fp32),
    }
    io = {}
    for name, (shp, dt) in shapes.items():
        io[name] = nc.dram_tensor(name, shp, dt, kind="ExternalInput").ap()
    io["out"] = nc.dram_tensor("out", [N, C], fp32, kind="ExternalOutput").ap()

    with tile.TileContext(nc) as tc:
        _emit(tc, io, gelu_mode=gelu_mode, mm_dt=mm_dt)
    nc.compile()
    return nc


def host_prep(inputs, mm_dt="bf16"):
    """Fold LN1/LN2 scale+shift into w_qk/w_v/w_fc1; build v-aug layout;
    cast weights to bf16."""
    import ml_dtypes
    f32 = np.float32
    bf = ml_dtypes.bfloat16

    x = np.asarray(inputs["x"], f32)
    w_qkv = np.asarray(inputs["w_qkv"], f32)
    b_qkv = np.asarray(inputs["b_qkv"], f32)
    ln1_w = np.asarray(inputs["ln1_w"], f32)
    ln1_b = np.asarray(inputs["ln1_b"], f32)
    ln2_w = np.asarray(inputs["ln2_w"], f32)
    ln2_b = np.asarray(inputs["ln2_b"], f32)

    w_q = w_qkv[:, 0:C]
    w_k = w_qkv[:, C:2 * C]
    w_v = w_qkv[:, 2 * C:3 * C]
    b_q = b_qkv[0:C]
    b_k = b_qkv[C:2 * C]
    b_v = b_qkv[2 * C:3 * C]

    # fold LN1: h = hn*ln1_w + ln1_b  =>  W' = ln1_w[:,None]*W, b' = b + W.T@ln1_b
    w_qk = np.concatenate([w_q, w_k], axis=1)          # [C, 2C]
    w_qk_f = ln1_w[:, None] * w_qk
    b_qk_f = np.concatenate([b_q, b_k]) + w_qk.T @ ln1_b

    w_v_f = ln1_w[:, None] * w_v
    b_v_f = b_v + w_v.T @ ln1_b
    w_v_aug = np.zeros((C, VW), f32)
    b_v_aug = np.zeros((VW,), f32)
    for h in range(H):
        w_v_aug[:, h * VB:h * VB + D] = w_v_f[:, h * D:(h + 1) * D]
        b_v_aug[h * VB:h * VB + D] = b_v_f[h * D:(h + 1) * D]
        b_v_aug[h * VB + D] = 1.0

    w_fc1 = np.asarray(inputs["w_fc1"], f32)
    b_fc1 = np.asarray(inputs["b_fc1"], f32)
    w_fc1_f = ln2_w[:, None] * w_fc1
    b_fc1_f = b_fc1 + w_fc1.T @ ln2_b

    def f8(a):
        # TRN e4m3 matches OCP e4m3fn bit-for-bit for |x| <= 240
        return np.ascontiguousarray(
            np.clip(a, -240, 240)).astype(ml_dtypes.float8_e4m3)

    common = {
        "w_qk": f8(w_qk_f),
        "b_qk": np.ascontiguousarray(b_qk_f, f32),
        "w_v_aug": f8(w_v_aug),
        "b_v_aug": b_v_aug,
        "w_proj": np.ascontiguousarray(
            np.asarray(inputs["w_proj"], f32)).astype(bf),
        "b_proj": np.ascontiguousarray(np.asarray(inputs["b_proj"], f32)),
        "w_fc1": np.ascontiguousarray(w_fc1_f).astype(bf),
        "b_fc1": np.ascontiguousarray(b_fc1_f, f32),
        "w_fc2": np.ascontiguousarray(np.asarray(inputs["w_fc2"], f32)).astype(bf),
        "b_fc2": np.ascontiguousarray(np.asarray(inputs["b_fc2"], f32)),
    }
    in_maps = []
    for i in range(x.shape[0]):
        m = dict(common)
        m["x"] = np.ascontiguousarray(x[i])
        in_maps.append(m)
    return in_maps


_CACHE = {}


def kernel(**inputs):
    from concourse.bass_utils import run_bass_kernel_spmd

    if "nc" not in _CACHE:
        _CACHE["nc"] = build_program(gelu_mode="hw")
    nc = _CACHE["nc"]
    in_maps = host_prep(inputs)
    res = run_bass_kernel_spmd(nc, in_maps, list(range(8)))
    out = np.stack([r["out"] for r in res.results], axis=0)
    return out.astype(np.float32)



# revision 24
# speedup vs baseline: 1.0021x; 1.0016x over previous
"""Transformer block (LN->MHA->residual->LN->MLP->residual) on 8 trn2 cores.

Data-parallel over batch: each of the 8 NeuronCores processes one [1024, 768]
batch element with the full weight set.  No collectives.

Tuned from the 467us baseline via trace analysis (final ~335-355us):
  - token-major for LN stats/residuals/output; feature-major (PE transposes)
    for matmul operands; one [P,1024] fp32 PSUM tag shared by every matmul
    phase so no pool boundary (= no PE drain stall) between attention, proj
    and MLP; attnv gets a private 2-buf PSUM pool.
  - x loaded as paired-tile DMAs on the sync queue ahead of everything
    (single-ring DMA triggers serialize ~2.6us/tile); w_qk transfers paced
    behind x via a gpsimd data dependency; w_fc1/w_fc2 loaded during proj.
  - column biases (b_qk, b_fc1) loaded as [m,128] rows and PE-transposed
    (a 4-byte-element gather DMA costs >20us); row biases applied via K=1
    ones-row matmul PSUM preload (v) or pre-added into the residual on
    idle DVE cycles (b_proj, b_fc2, in-place in x_sb).
  - LN mean via scalar-engine accum_out, sum(x^2) via DVE stt accum (LN1)
    or rides the residual-add accum (LN2); hn in bf16 directly; LN2 lagged
    4 tiles behind proj and its tail interleaved with fc1 slab-0 m-groups.
  - attention: scoresT = k^T.T @ q^T per head-half (bf16, PE row groups);
    score matmuls fill PSUM bank-pairs so each Exp covers [128,1024];
    E/v/h1 in fp8e4 with DoubleRow matmuls for qkv/v/attnv (halves the
    instruction count; NB DoubleRow streams at the same 1 col/cycle as
    bf16 on this HW, so it only pays off where per-instruction overhead is
    exposed); softmax denominator via ones-slot in the 68-wide padded
    v-aug blocks (16B stride alignment required by dual-fp8 LDWEIGHTS);
    1/den via vector reciprocal from a partition-0 r tile (the custom-DVE
    reciprocal_approx ISA ops produce garbage on this HW, and ISA ops
    mishandle nonzero partition offsets); attnv halves interleaved between
    score halves so the normalize chain hides under the next half.
  - k feature-major tiles computed one slab ahead with the bias-cast on
    DVE (emitted before the reciprocals so the queue never blocks them);
    q bias-cast on scalar (Identity shares the Exp act table); sqrt table
    pre-warmed during the DMA preamble.
  - fc1 m-outer over [128,1024] 2-slab psum slots was reverted to
    slab-split so slab 0 starts before LN2 of tiles 4..7; fc2 bf16 (fp8
    in the MLP costs ~2e-2 rel err, over budget; attention fp8 costs
    ~5e-4).
  - weights bf16/fp8e4 cast on host with LN scale/shift folded in;
    accumulation and the residual path in fp32.
"""

import numpy as np

P = 128
N = 1024          # tokens per core
C = 768           # embed
H = 12            # heads
D = 64            # head dim
HID = 3072
NT = N // P       # 8 token tiles
CK = C // P       # 6 feature k-tiles
HK = HID // P     # 24
VB = 68           # padded per-head v block (16B-friendly)
VW = H * VB       # 816 = v-aug width
EPS = 1e-5
NSLABS = 2
NSL = N // NSLABS  # 512
HPAIRS = H // 2    # 6


def _emit(tc, io, gelu_mode="hw", mm_dt="bf16"):
    """Emit the whole block into TileContext tc. io: dict name->AP."""
    from contextlib import ExitStack

    from concourse import mybir
    from concourse.masks import make_identity

    nc = tc.nc
    fp32 = mybir.dt.float32
    bf16 = mybir.dt.bfloat16
    f8 = mybir.dt.float8e4
    DR = mybir.MatmulPerfMode.DoubleRow
    AF = mybir.ActivationFunctionType
    ALU = mybir.AluOpType

    with ExitStack() as ctx:
        const = ctx.enter_context(tc.tile_pool(name="const", bufs=1))
        work = ctx.enter_context(tc.tile_pool(name="work", bufs=2))
        xpool = ctx.enter_context(tc.tile_pool(name="xpool", bufs=1))
        h2Tpool = ctx.enter_context(tc.tile_pool(name="h2Tpool", bufs=1))
        wprojp = ctx.enter_context(tc.tile_pool(name="wproj", bufs=1))
        # one [P,1024] fp32 psum tag shared by every matmul phase: no pool
        # boundary between attention / proj / MLP means no PE drain stalls
        psM = ctx.enter_context(tc.tile_pool(name="psM", bufs=3, space="PSUM"))

        # x tiles: paired-tile DMAs, one ring trigger per 2 tiles, spread
        # across FOUR engine queues -- a single trigger costs ~2.1us of the
        # issuing engine's time, so serializing all four on one ring delays
        # the data ~8us
        xq = [xpool.tile([P, 2, C], fp32, tag=f"xq_{u}", name=f"xq_{u}")
              for u in range(NT // 2)]
        x_sb = [xq[t // 2][:, t % 2, :] for t in range(NT)]
        # only SP (sync), Activation (scalar) and gpsimd can initiate DMAs;
        # two rings x two triggers each still beats four serialized triggers
        xqueues = [nc.sync, nc.scalar, nc.sync, nc.scalar]
        for u in range(NT // 2):
            xqueues[u].dma_start(
                out=xq[u],
                in_=io["x"][2 * u * P:(2 * u + 2) * P, :].rearrange(
                    "(i p) c -> p i c", p=P))

        # identity matrices (gpsimd, cheap, needed by transposes)
        ident = const.tile([P, P], bf16, tag="ident", name="ident")
        make_identity(nc, ident)
        ident32 = const.tile([HK, HK], fp32, tag="ident32", name="ident32")
        make_identity(nc, ident32)
        eps_t = const.tile([P, 1], fp32, tag="eps", name="eps")
        nc.vector.memset(eps_t, EPS)
        warm = work.tile([P, 1], fp32, tag="warm", name="warm", bufs=1)
        nc.scalar.activation(out=warm, in_=eps_t, func=AF.Sqrt,
                             bias=eps_t, scale=1.0)

        def row_bcast(nm, width):
            row = const.tile([1, width], fp32, tag=f"r_{nm}", name=f"r_{nm}")
            nc.gpsimd.dma_start(
                out=row, in_=io[nm].rearrange("(a w) -> a w", a=1))
            t = const.tile([P, width], fp32, tag=f"bc_{nm}", name=f"bc_{nm}")
            nc.gpsimd.partition_broadcast(t, row)
            return t

        ones_row = const.tile([1, P], bf16, tag="ones_row", name="ones_row")
        nc.vector.memset(ones_row, 1.0)
        b_v_row = const.tile([1, VW], bf16, tag="bvrow", name="b_v_row")
        nc.gpsimd.dma_start(
            out=b_v_row, in_=io["b_v_aug"].rearrange("(a w) -> a w", a=1))

        # col-bias sources [m, 128] (transposed on PE once tiles land);
        # DMAs issued inside phase 1 AFTER the w_v transfers (V matmuls
        # need w_v first; these only feed idle-PE transposes)
        bqk_src = const.tile([2 * CK, P], fp32, tag="bqk_src", name="bqk_src")
        bfc1_src = const.tile([HK, P], fp32, tag="bfc1_src", name="bfc1_src")
        b_qk_col = const.tile([P, 2 * CK], fp32, tag="bqk_col", name="b_qk_col")
        b_fc1_col = const.tile([P, HK], fp32, tag="bfc1_col", name="b_fc1_col")

        # h2T: one [P, CK, N] tile -> LN2 transposes land with ONE wide copy
        h2T = h2Tpool.tile([P, CK, N], bf16, tag="h2T", name="h2T")

        w_proj_sb = []
        for k in range(CK):
            wt = wprojp.tile([P, C], bf16, tag=f"wp_{k}", name=f"wp_{k}")
            w_proj_sb.append(wt)

        # ---------------- LN helper (scalar-engine stats) ----------------
        def ln_stats_hn(src, sum_in=None):
            """mean/var stats split across Scalar/DVE; hn on gpsimd."""
            if sum_in is None:
                scr = work.tile([P, C], f8, tag="scr", name="scr", bufs=1)
                mean = work.tile([P, 1], fp32, tag="s1", name="s1")
                nc.scalar.activation(out=scr, in_=src, func=AF.Copy,
                                     scale=1.0 / C, accum_out=mean)
                # sum of squares on DVE (stt accum), scaled later
                scr2 = work.tile([P, C], fp32, tag="scrv", name="scrv",
                                 bufs=1)
                s2r = work.tile([P, 1], fp32, tag="s2", name="s2")
                nc.vector.scalar_tensor_tensor(
                    out=scr2, in0=src, scalar=1.0, in1=src,
                    op0=ALU.mult, op1=ALU.mult, accum_out=s2r)
                s2scale = 1.0 / C
            else:
                mean = work.tile([P, 1], fp32, tag="s1", name="s1")
                nc.vector.tensor_scalar(out=mean, in0=sum_in,
                                        scalar1=1.0 / C, scalar2=None,
                                        op0=ALU.mult)
                scr2 = work.tile([P, C], f8, tag="scr", name="scr2", bufs=1)
                s2r = work.tile([P, 1], fp32, tag="s2", name="s2")
                nc.scalar.activation(out=scr2, in_=src, func=AF.Square,
                                     scale=(1.0 / C) ** 0.5, accum_out=s2r)
                s2scale = 1.0
            m2 = work.tile([P, 1], fp32, tag="m2", name="m2")
            nc.vector.tensor_mul(out=m2, in0=mean, in1=mean)
            vv = work.tile([P, 1], fp32, tag="vv", name="vv")
            nc.vector.scalar_tensor_tensor(
                out=vv, in0=s2r, scalar=s2scale, in1=m2,
                op0=ALU.mult, op1=ALU.subtract)
            std = work.tile([P, 1], fp32, tag="std", name="std")
            nc.scalar.activation(out=std, in_=vv, func=AF.Sqrt,
                                 bias=eps_t, scale=1.0)
            istd = work.tile([P, 1], fp32, tag="istd", name="istd")
            nc.vector.reciprocal(out=istd, in_=std)
            hn = work.tile([P, C], bf16, tag="hn", name="hn")
            nc.vector.tensor_scalar(out=hn, in0=src,
                                    scalar1=mean, scalar2=istd,
                                    op0=ALU.subtract, op1=ALU.mult)
            return hn

        def transpose_tile(hn, psTpool, dstT, t, eng):
            """6 transposes into one psum bank + one wide strided copy."""
            pt6 = psTpool.tile([P, C], bf16, tag="tr", name="tr")
            for c in range(CK):
                nc.tensor.transpose(pt6[:, c * P:(c + 1) * P],
                                    hn[:, c * P:(c + 1) * P], ident)
            half = CK // 2
            src = pt6.rearrange("p (c q) -> p c q", c=CK)
            dst_lo = dstT[:, :half, t * P:(t + 1) * P]
            dst_hi = dstT[:, half:, t * P:(t + 1) * P]
            if eng == "s":
                nc.scalar.copy(out=dst_lo, in_=src[:, :half, :])
                nc.vector.tensor_copy(out=dst_hi, in_=src[:, half:, :])
            else:
                nc.vector.tensor_copy(out=dst_lo, in_=src[:, :half, :])
                nc.scalar.copy(out=dst_hi, in_=src[:, half:, :])

        with tc.tile_pool(name="aopool", bufs=1) as aopool:
            attn_oT = [aopool.tile([P, N], bf16, tag=f"aoT_{c}",
                                   name=f"aoT_{c}") for c in range(CK)]

            with tc.tile_pool(name="wqk", bufs=1) as wqk_pool, \
                 tc.tile_pool(name="h1Tpool", bufs=1) as h1Tpool, \
                 tc.tile_pool(name="vpool", bufs=1) as vpool:

                w_qk_sb = []
                for j in range(CK // 2):
                    wt = wqk_pool.tile([P, 2, 2 * C], f8, tag=f"wqk_{j}",
                                       name=f"wqk_{j}")
                    w_qk_sb.append(wt)
                h1T = h1Tpool.tile([P, CK, N], f8, tag="h1T", name="h1T")
                v_sb = vpool.tile([P, NT, VW], f8, tag="v_sb", name="v_sb")

                # ---------- phase 1: LN1 + V ----------
                with tc.tile_pool(name="wv", bufs=1) as wv_pool, \
                     tc.tile_pool(name="psT", bufs=2, space="PSUM") as psT:
                    # w_v pair-interleaved on host ([p, c, i] with the
                    # K-pair adjacent) so dual-fp8 DR streams 2 vals/cycle
                    w_v_sb = []
                    for j in range(CK // 2):
                        wt = wv_pool.tile([P, VW, 2], f8, tag=f"wv_{j}",
                                          name=f"wv_{j}")
                        nc.gpsimd.dma_start(
                            out=wt,
                            in_=io["w_v_il"][j * P:(j + 1) * P,
                                             :].rearrange(
                                "p (c i) -> p c i", i=2))
                        w_v_sb.append(wt)
                    nc.gpsimd.dma_start(
                        out=bqk_src,
                        in_=io["b_qk"].rearrange("(o p) -> o p", p=P))
                    nc.gpsimd.dma_start(
                        out=bfc1_src,
                        in_=io["b_fc1"].rearrange("(o p) -> o p", p=P))

                    # col-bias transposes (PE idles here anyway)
                    ptb = psT.tile([P, HK], fp32, tag="tr", name="ptb")
                    nc.tensor.transpose(ptb[:, :2 * CK], bqk_src,
                                        ident32[:2 * CK, :2 * CK])
                    nc.vector.tensor_copy(out=b_qk_col, in_=ptb[:, :2 * CK])
                    ptb2 = psT.tile([P, HK], fp32, tag="tr", name="ptb2")
                    nc.tensor.transpose(ptb2, bfc1_src, ident32)
                    nc.vector.tensor_copy(out=b_fc1_col, in_=ptb2)

                    for t in range(NT):
                        hn = ln_stats_hn(x_sb[t])
                        transpose_tile(hn, psT, h1T, t,
                                       "s" if t % 2 else "v")
                        ps = psM.tile([P, 2 * NSL], fp32, tag="mm",
                                      name="mm")
                        for off, w in ((0, NSL), (NSL, VW - NSL)):
                            nc.tensor.matmul(
                                ps[:, off:off + w], ones_row,
                                b_v_row[:, off:off + w],
                                start=True, stop=False)
                            for j in range(CK // 2):
                                nc.tensor.matmul(
                                    ps[:, off:off + w],
                                    h1T[:, 2 * j:2 * j + 2,
                                        t * P:(t + 1) * P],
                                    w_v_sb[j][:, off:off + w, :].rearrange(
                                        "p c i -> p i c"),
                                    start=False, stop=(j == CK // 2 - 1),
                                    perf_mode=DR)
                        nc.scalar.copy(out=v_sb[:, t, :VW // 2],
                                       in_=ps[:, :VW // 2])
                        nc.vector.tensor_copy(out=v_sb[:, t, VW // 2:VW],
                                              in_=ps[:, VW // 2:VW])
                        if 3 <= t < 3 + CK // 2:
                            # wqk transfers deferred so x tiles keep full
                            # DMA bandwidth: pace the gpsimd queue with a
                            # tiny dep on this tile's v output first
                            j = t - 3
                            pace = work.tile([1, 1], f8, tag="pace",
                                             name="pace")
                            nc.gpsimd.tensor_copy(
                                out=pace, in_=v_sb[0:1, t, 0:1])
                            nc.gpsimd.dma_start(
                                out=w_qk_sb[j],
                                in_=io["w_qk"][2 * j * P:(2 * j + 2) * P,
                                               :].rearrange(
                                    "(i p) c -> p i c", p=P))

                # ---------- phase 2+3: attention ----------
                b_proj_bc = row_bcast("b_proj", C)
                for k in range(CK):
                    nc.sync.dma_start(
                        out=w_proj_sb[k],
                        in_=io["w_proj"][k * P:(k + 1) * P, :])
                with tc.tile_pool(name="kqpool", bufs=2) as kqpool, \
                     tc.tile_pool(name="epool", bufs=3) as epool, \
                     tc.tile_pool(name="rpool", bufs=4) as rpool, \
                     tc.tile_pool(name="psV", bufs=2, space="PSUM") as psV:

                    def emit_k(hp):
                        ps = psM.tile([P, N], fp32, tag="mm", name="mm")
                        for ns in range(NSLABS):
                            sl = slice(ns * NSL, (ns + 1) * NSL)
                            for j in range(CK // 2):
                                nc.tensor.matmul(
                                    ps[:, sl],
                                    w_qk_sb[j][:, :, C + hp * P:
                                               C + (hp + 1) * P],
                                    h1T[:, 2 * j:2 * j + 2, sl],
                                    start=(j == 0), stop=(j == CK // 2 - 1),
                                    perf_mode=DR)
                        kt = kqpool.tile([P, N], bf16, tag="ksb", name="ksb")
                        nc.vector.tensor_scalar(
                            out=kt, in0=ps,
                            scalar1=b_qk_col[:, CK + hp:CK + hp + 1],
                            scalar2=None, op0=ALU.add)
                        return kt

                    def emit_q(hp, ns):
                        ps = psM.tile([P, N], fp32, tag="mm", name="mm")
                        sl = slice(ns * NSL, (ns + 1) * NSL)
                        for j in range(CK // 2):
                            nc.tensor.matmul(
                                ps[:, :NSL],
                                w_qk_sb[j][:, :, hp * P:(hp + 1) * P],
                                h1T[:, 2 * j:2 * j + 2, sl],
                                start=(j == 0), stop=(j == CK // 2 - 1),
                                perf_mode=DR)
                        qt = kqpool.tile([P, NSL], bf16, tag="qsb",
                                         name="qsb")
                        # bias-cast on DVE: keeps Scalar exclusively on Exp
                        nc.vector.tensor_scalar(
                            out=qt, in0=ps[:, :NSL],
                            scalar1=b_qk_col[:, hp:hp + 1],
                            scalar2=None, op0=ALU.add)
                        return qt

                    def emit_scores_pair(kt, qt, j, E):
                        # both head-halves' K=64 score matmuls issued
                        # back-to-back: lhsT base partitions 0/64 map to
                        # disjoint PE row-groups, so they run CONCURRENTLY
                        pss = [psM.tile([P, N], fp32, tag="mm", name="mm")
                               for _ in range(2)]
                        for i in range(2):
                            mt = 2 * j + i
                            for half in range(2):
                                pr = slice(half * D, (half + 1) * D)
                                nc.tensor.matmul(
                                    pss[half][:, i * NSL:(i + 1) * NSL],
                                    kt[pr, mt * P:(mt + 1) * P],
                                    qt[pr, :],
                                    start=True, stop=True)
                        for half in range(2):
                            # pair-interleaved [p, n, i]: the two k-tiles'
                            # values for a query adjacent -> dual-fp8 DR
                            # attnv streams 2 vals/cycle
                            e = epool.tile([P, NSL, 2], f8,
                                           tag=f"E_{half}_{j}",
                                           name=f"E_{half}_{j}")
                            nc.scalar.activation(
                                out=e.rearrange("p n i -> p i n"),
                                in_=pss[half].rearrange(
                                    "p (i n) -> p i n", i=2),
                                func=AF.Exp, scale=0.125)
                            E[(half, j)] = e

                    # normalize chain split into three stages so the DVE's
                    # strict-FIFO queue never blocks on a gpsimd round-trip:
                    # mm+recip first, both bcasts together, mults at the
                    # END of the instance (bcasts are long done by then)
                    def attnv_mm(hp, ns, E, half):
                        h = 2 * hp + half
                        ps_o = psV.tile([P, NSL], fp32, tag="vo",
                                        name="vo")[:D + 1, :]
                        for u in range(NT // 2):
                            e = E[(half, u)]
                            nc.tensor.matmul(
                                ps_o,
                                v_sb[:, 2 * u:2 * u + 2,
                                     h * VB:h * VB + D + 1],
                                e.rearrange("p n i -> p i n"),
                                start=(u == 0), stop=(u == NT // 2 - 1),
                                perf_mode=DR)
                        # fast custom-DVE reciprocal: 5x cheaper than
                        # InstReciprocal -- but it computes garbage for
                        # nonzero partition bases (verified: NaN), so
                        # stage den from partition 64 to 0 first
                        dn = rpool.tile([1, NSL], fp32, tag="dn",
                                        name="dn")
                        nc.vector.tensor_copy(
                            out=dn, in_=ps_o[D:D + 1, :])
                        r = rpool.tile([1, NSL], fp32, tag="r",
                                       name="r")
                        nc.vector.reciprocal_approx_fast(out=r, in_=dn)
                        return [hp, ns, half, ps_o, r]

                    def attnv_bcast(a):
                        rb = rpool.tile([D, NSL], fp32, tag="rb",
                                        name="rb")
                        nc.gpsimd.partition_broadcast(rb, a[4])
                        a.append(rb)

                    def attnv_mult(a):
                        hp, ns, half, ps_o, r, rb = a
                        nsl = slice(ns * NSL, (ns + 1) * NSL)
                        nc.vector.tensor_mul(
                            out=attn_oT[hp][half * D:(half + 1) * D, nsl],
                            in0=ps_o[:D, :], in1=rb)

                    prev = None
                    x2q = list(range(NT))  # b_proj pre-add (in-place x)
                    kt_cur = emit_k(0)
                    kt_next = None
                    for hp in range(HPAIRS):
                        for ns in range(NSLABS):
                            qt = emit_q(hp, ns)
                            E = {}
                            emit_scores_pair(kt_cur, qt, 0, E)
                            a0 = attnv_mm(*prev, 0) if prev else None
                            emit_scores_pair(kt_cur, qt, 1, E)
                            if prev is not None:
                                a1 = attnv_mm(*prev, 1)
                                attnv_bcast(a0)
                                attnv_bcast(a1)
                            if ns == 0 and hp + 1 < HPAIRS:
                                kt_next = emit_k(hp + 1)
                            emit_scores_pair(kt_cur, qt, 2, E)
                            emit_scores_pair(kt_cur, qt, 3, E)
                            if prev is not None:
                                attnv_mult(a0)
                                attnv_mult(a1)
                            if x2q:
                                # DVE (has slack): gpsimd's strict FIFO
                                # would stall the partition_broadcasts
                                t = x2q.pop(0)
                                nc.vector.tensor_add(
                                    out=x_sb[t], in0=x_sb[t],
                                    in1=b_proj_bc)
                            prev = (hp, ns, E)
                        kt_cur = kt_next
                    a0 = attnv_mm(*prev, 0)
                    a1 = attnv_mm(*prev, 1)
                    attnv_bcast(a0)
                    attnv_bcast(a1)
                    attnv_mult(a0)
                    attnv_mult(a1)

            # ---------- phase 4+5+6: proj + LN2 (lagged) + MLP ----------
            b_fc2_bc = row_bcast("b_fc2", C)
            with tc.tile_pool(name="wfc1", bufs=1) as w1pool, \
                 tc.tile_pool(name="wfc2", bufs=1) as w2pool, \
                 tc.tile_pool(name="psT2", bufs=2, space="PSUM") as psT2:
                w1 = []
                for k in range(CK):
                    wt = w1pool.tile([P, HID], bf16, tag=f"wfc1_{k}",
                                     name=f"wfc1_{k}")
                    nc.sync.dma_start(
                        out=wt, in_=io["w_fc1"][k * P:(k + 1) * P, :])
                    w1.append(wt)
                w2g = []
                for g in range(CK):
                    wt = w2pool.tile([P, HK // CK, C], bf16, tag=f"wfc2_{g}",
                                     name=f"wfc2_{g}")
                    nc.sync.dma_start(
                        out=wt,
                        in_=io["w_fc2"][g * 512:(g + 1) * 512, :].rearrange(
                            "(o p) c -> p o c", p=P))
                    w2g.append(wt)

                def emit_proj(t):
                    ps = psM.tile([P, 2 * NSL], fp32, tag="mm", name="mm")
                    for off, w in ((0, NSL), (NSL, C - NSL)):
                        for k in range(CK):
                            nc.tensor.matmul(
                                ps[:, off:off + w],
                                attn_oT[k][:, t * P:(t + 1) * P],
                                w_proj_sb[k][:, off:off + w],
                                start=(k == 0), stop=(k == CK - 1))
                    s1raw = work.tile([P, 1], fp32, tag="s1r", name="s1r")
                    nc.vector.scalar_tensor_tensor(
                        out=x_sb[t], in0=ps[:, :C], scalar=1.0,
                        in1=x_sb[t], op0=ALU.mult, op1=ALU.add,
                        accum_out=s1raw)
                    return s1raw

                def emit_ln2(t, s1raw):
                    hn = ln_stats_hn(x_sb[t], sum_in=s1raw)
                    transpose_tile(hn, psT2, h2T, t, "s" if t % 2 else "v")

                LAG = 2
                s1s = {}
                for t in range(NT):
                    s1s[t] = emit_proj(t)
                    if t >= LAG:
                        emit_ln2(t - LAG, s1s.pop(t - LAG))

                gelu_f = AF.Gelu if gelu_mode == "hw" else AF.Identity
                with tc.tile_pool(name="gpool", bufs=1) as gpool, \
                     tc.tile_pool(name="opool", bufs=2) as opool:
                    gT = gpool.tile([P, HK, N], bf16, tag="gT", name="gT")
                    x2q = list(range(NT))  # b_fc2 pre-add (in-place x)

                    def emit_fc1(ns, ms):
                        sl = slice(ns * NSL, (ns + 1) * NSL)
                        for m in ms:
                            ps = psM.tile([P, 2 * NSL], fp32, tag="mm",
                                          name="mm")
                            for k in range(CK):
                                nc.tensor.matmul(
                                    ps[:, :NSL],
                                    w1[k][:, m * P:(m + 1) * P],
                                    h2T[:, k, sl],
                                    start=(k == 0), stop=(k == CK - 1))
                            nc.scalar.activation(
                                out=gT[:, m, sl], in_=ps[:, :NSL],
                                func=gelu_f,
                                bias=b_fc1_col[:, m:m + 1], scale=1.0)

                    # LN2 tail FIRST (all Sqrts contiguous -> one act-table
                    # load), then all fc1 gelus (one gelu table load); the
                    # baseline interleave thrashed 5 table loads (~13us of
                    # Scalar time in the fc1-slab-0 window)
                    for t in range(NT - LAG, NT):
                        emit_ln2(t, s1s.pop(t))
                    for i in range(4):
                        emit_fc1(0, range(i * 6, (i + 1) * 6))
                        if x2q:
                            tt = x2q.pop(0)
                            nc.gpsimd.tensor_add(
                                out=x_sb[tt], in0=x_sb[tt], in1=b_fc2_bc)
                    for i in range(4):
                        emit_fc1(1, range(i * 6, (i + 1) * 6))
                        if x2q:
                            tt = x2q.pop(0)
                            nc.gpsimd.tensor_add(
                                out=x_sb[tt], in0=x_sb[tt], in1=b_fc2_bc)
                    while x2q:
                        tt = x2q.pop(0)
                        nc.vector.tensor_add(
                            out=x_sb[tt], in0=x_sb[tt], in1=b_fc2_bc)

                    for t in range(NT):
                        ps = psM.tile([P, 2 * NSL], fp32, tag="mm",
                                      name="mm")
                        ot = opool.tile([P, C], fp32, tag="ot", name="ot")
                        for off, w in ((0, NSL), (NSL, C - NSL)):
                            for k in range(HK):
                                nc.tensor.matmul(
                                    ps[:, off:off + w],
                                    gT[:, k, t * P:(t + 1) * P],
                                    w2g[k // 4][:, k % 4, off:off + w],
                                    start=(k == 0), stop=(k == HK - 1))
                            nc.vector.tensor_add(
                                out=ot[:, off:off + w],
                                in0=ps[:, off:off + w],
                                in1=x_sb[t][:, off:off + w])
                            nc.sync.dma_start(
                                out=io["out"][t * P:(t + 1) * P,
                                              off:off + w],
                                in_=ot[:, off:off + w])


def build_program(gelu_mode="hw", mm_dt="bf16"):
    import concourse.tile as tile
    from concourse import bacc, mybir

    fp32 = mybir.dt.float32
    bf16 = mybir.dt.bfloat16
    nc = bacc.Bacc("TRN2", target_bir_lowering=False, debug=False,
                   num_devices=8)

    f8 = mybir.dt.float8e4
    shapes = {
        "x": ([N, C], fp32),
        "w_qk": ([C, 2 * C], f8), "b_qk": ([2 * C], fp32),
        "w_v_il": ([C // 2, 2 * VW], f8), "b_v_aug": ([VW], fp32),
        "w_proj": ([C, C], bf16), "b_proj": ([C], fp32),
        "w_fc1": ([C, HID], bf16), "b_fc1": ([HID], fp32),
        "w_fc2": ([HID, C], bf16), "b_fc2": ([C], fp32),
    }
    io = {}
    for name, (shp, dt) in shapes.items():
        io[name] = nc.dram_tensor(name, shp, dt, kind="ExternalInput").ap()
    io["out"] = nc.dram_tensor("out", [N, C], fp32, kind="ExternalOutput").ap()

    with tile.TileContext(nc) as tc:
        _emit(tc, io, gelu_mode=gelu_mode, mm_dt=mm_dt)
    nc.compile()
    return nc


def host_prep(inputs, mm_dt="bf16"):
    """Fold LN1/LN2 scale+shift into w_qk/w_v/w_fc1; build v-aug layout;
    cast weights to bf16."""
    import ml_dtypes
    f32 = np.float32
    bf = ml_dtypes.bfloat16

    x = np.asarray(inputs["x"], f32)
    w_qkv = np.asarray(inputs["w_qkv"], f32)
    b_qkv = np.asarray(inputs["b_qkv"], f32)
    ln1_w = np.asarray(inputs["ln1_w"], f32)
    ln1_b = np.asarray(inputs["ln1_b"], f32)
    ln2_w = np.asarray(inputs["ln2_w"], f32)
    ln2_b = np.asarray(inputs["ln2_b"], f32)

    w_q = w_qkv[:, 0:C]
    w_k = w_qkv[:, C:2 * C]
    w_v = w_qkv[:, 2 * C:3 * C]
    b_q = b_qkv[0:C]
    b_k = b_qkv[C:2 * C]
    b_v = b_qkv[2 * C:3 * C]

    # fold LN1: h = hn*ln1_w + ln1_b  =>  W' = ln1_w[:,None]*W, b' = b + W.T@ln1_b
    w_qk = np.concatenate([w_q, w_k], axis=1)          # [C, 2C]
    w_qk_f = ln1_w[:, None] * w_qk
    b_qk_f = np.concatenate([b_q, b_k]) + w_qk.T @ ln1_b

    w_v_f = ln1_w[:, None] * w_v
    b_v_f = b_v + w_v.T @ ln1_b
    w_v_aug = np.zeros((C, VW), f32)
    b_v_aug = np.zeros((VW,), f32)
    for h in range(H):
        w_v_aug[:, h * VB:h * VB + D] = w_v_f[:, h * D:(h + 1) * D]
        b_v_aug[h * VB:h * VB + D] = b_v_f[h * D:(h + 1) * D]
        b_v_aug[h * VB + D] = 1.0

    w_fc1 = np.asarray(inputs["w_fc1"], f32)
    b_fc1 = np.asarray(inputs["b_fc1"], f32)
    w_fc1_f = ln2_w[:, None] * w_fc1
    b_fc1_f = b_fc1 + w_fc1.T @ ln2_b

    def f8(a):
        # TRN e4m3 matches OCP e4m3fn bit-for-bit for |x| <= 240
        return np.ascontiguousarray(
            np.clip(a, -240, 240)).astype(ml_dtypes.float8_e4m3)

    # pair-interleave w_v_aug for dual-fp8 DR: row (j*128+p) holds
    # [c, i] flattened where i picks feature block 2j+i
    w_v_il = np.zeros((C // 2, VW, 2), f32)
    for j in range(C // 256):
        for i in range(2):
            w_v_il[j * 128:(j + 1) * 128, :, i] = \
                w_v_aug[(2 * j + i) * 128:(2 * j + i + 1) * 128, :]
    common = {
        "w_qk": f8(w_qk_f),
        "b_qk": np.ascontiguousarray(b_qk_f, f32),
        "w_v_il": f8(w_v_il.reshape(C // 2, 2 * VW)),
        "b_v_aug": b_v_aug,
        "w_proj": np.ascontiguousarray(
            np.asarray(inputs["w_proj"], f32)).astype(bf),
        "b_proj": np.ascontiguousarray(np.asarray(inputs["b_proj"], f32)),
        "w_fc1": np.ascontiguousarray(w_fc1_f).astype(bf),
        "b_fc1": np.ascontiguousarray(b_fc1_f, f32),
        "w_fc2": np.ascontiguousarray(np.asarray(inputs["w_fc2"], f32)).astype(bf),
        "b_fc2": np.ascontiguousarray(np.asarray(inputs["b_fc2"], f32)),
    }
    in_maps = []
    for i in range(x.shape[0]):
        m = dict(common)
        m["x"] = np.ascontiguousarray(x[i])
        in_maps.append(m)
    return in_maps


_CACHE = {}


def kernel(**inputs):
    from concourse.bass_utils import run_bass_kernel_spmd

    if "nc" not in _CACHE:
        _CACHE["nc"] = build_program(gelu_mode="hw")
    nc = _CACHE["nc"]
    in_maps = host_prep(inputs)
    res = run_bass_kernel_spmd(nc, in_maps, list(range(8)))
    out = np.stack([r["out"] for r in res.results], axis=0)
    return out.astype(np.float32)



# revision 29
# speedup vs baseline: 1.0287x; 1.0265x over previous
"""Transformer block (LN->MHA->residual->LN->MLP->residual) on 8 trn2 cores.

Data-parallel over batch: each of the 8 NeuronCores processes one [1024, 768]
batch element with the full weight set.  No collectives.

Tuned from the 467us baseline via trace analysis (final ~335-355us):
  - token-major for LN stats/residuals/output; feature-major (PE transposes)
    for matmul operands; one [P,1024] fp32 PSUM tag shared by every matmul
    phase so no pool boundary (= no PE drain stall) between attention, proj
    and MLP; attnv gets a private 2-buf PSUM pool.
  - x loaded as paired-tile DMAs on the sync queue ahead of everything
    (single-ring DMA triggers serialize ~2.6us/tile); w_qk transfers paced
    behind x via a gpsimd data dependency; w_fc1/w_fc2 loaded during proj.
  - column biases (b_qk, b_fc1) loaded as [m,128] rows and PE-transposed
    (a 4-byte-element gather DMA costs >20us); row biases applied via K=1
    ones-row matmul PSUM preload (v) or pre-added into the residual on
    idle DVE cycles (b_proj, b_fc2, in-place in x_sb).
  - LN mean via scalar-engine accum_out, sum(x^2) via DVE stt accum (LN1)
    or rides the residual-add accum (LN2); hn in bf16 directly; LN2 lagged
    4 tiles behind proj and its tail interleaved with fc1 slab-0 m-groups.
  - attention: scoresT = k^T.T @ q^T per head-half (bf16, PE row groups);
    score matmuls fill PSUM bank-pairs so each Exp covers [128,1024];
    E/v/h1 in fp8e4 with DoubleRow matmuls for qkv/v/attnv (halves the
    instruction count; NB DoubleRow streams at the same 1 col/cycle as
    bf16 on this HW, so it only pays off where per-instruction overhead is
    exposed); softmax denominator via ones-slot in the 68-wide padded
    v-aug blocks (16B stride alignment required by dual-fp8 LDWEIGHTS);
    1/den via vector reciprocal from a partition-0 r tile (the custom-DVE
    reciprocal_approx ISA ops produce garbage on this HW, and ISA ops
    mishandle nonzero partition offsets); attnv halves interleaved between
    score halves so the normalize chain hides under the next half.
  - k feature-major tiles computed one slab ahead with the bias-cast on
    DVE (emitted before the reciprocals so the queue never blocks them);
    q bias-cast on scalar (Identity shares the Exp act table); sqrt table
    pre-warmed during the DMA preamble.
  - fc1 m-outer over [128,1024] 2-slab psum slots was reverted to
    slab-split so slab 0 starts before LN2 of tiles 4..7; fc2 bf16 (fp8
    in the MLP costs ~2e-2 rel err, over budget; attention fp8 costs
    ~5e-4).
  - weights bf16/fp8e4 cast on host with LN scale/shift folded in;
    accumulation and the residual path in fp32.
"""

import numpy as np

P = 128
N = 1024          # tokens per core
C = 768           # embed
H = 12            # heads
D = 64            # head dim
HID = 3072
NT = N // P       # 8 token tiles
CK = C // P       # 6 feature k-tiles
HK = HID // P     # 24
VB = 68           # padded per-head v block (16B-friendly)
VW = H * VB       # 816 = v-aug width
EPS = 1e-5
NSLABS = 2
NSL = N // NSLABS  # 512
HPAIRS = H // 2    # 6


def _emit(tc, io, gelu_mode="hw", mm_dt="bf16"):
    """Emit the whole block into TileContext tc. io: dict name->AP."""
    from contextlib import ExitStack

    from concourse import mybir
    from concourse.masks import make_identity

    nc = tc.nc
    fp32 = mybir.dt.float32
    bf16 = mybir.dt.bfloat16
    f8 = mybir.dt.float8e4
    DR = mybir.MatmulPerfMode.DoubleRow
    AF = mybir.ActivationFunctionType
    ALU = mybir.AluOpType

    with ExitStack() as ctx:
        const = ctx.enter_context(tc.tile_pool(name="const", bufs=1))
        work = ctx.enter_context(tc.tile_pool(name="work", bufs=2))
        xpool = ctx.enter_context(tc.tile_pool(name="xpool", bufs=1))
        h2Tpool = ctx.enter_context(tc.tile_pool(name="h2Tpool", bufs=1))
        wprojp = ctx.enter_context(tc.tile_pool(name="wproj", bufs=1))
        # one [P,1024] fp32 psum tag shared by every matmul phase: no pool
        # boundary between attention / proj / MLP means no PE drain stalls
        psM = ctx.enter_context(tc.tile_pool(name="psM", bufs=3, space="PSUM"))

        # x tiles: paired-tile DMAs (one ring trigger per 2 tiles) on the
        # sync queue first -- per-tile triggers serialize ~2.6us/tile on a
        # single ring
        xq = [xpool.tile([P, 2, C], fp32, tag=f"xq_{u}", name=f"xq_{u}")
              for u in range(NT // 2)]
        x_sb = [xq[t // 2][:, t % 2, :] for t in range(NT)]
        for u in range(NT // 2):
            nc.sync.dma_start(
                out=xq[u],
                in_=io["x"][2 * u * P:(2 * u + 2) * P, :].rearrange(
                    "(i p) c -> p i c", p=P))

        # identity matrices (gpsimd, cheap, needed by transposes)
        ident = const.tile([P, P], bf16, tag="ident", name="ident")
        make_identity(nc, ident)
        ident32 = const.tile([HK, HK], fp32, tag="ident32", name="ident32")
        make_identity(nc, ident32)
        eps_t = const.tile([P, 1], fp32, tag="eps", name="eps")
        nc.vector.memset(eps_t, EPS)
        warm = work.tile([P, 1], fp32, tag="warm", name="warm", bufs=1)
        nc.scalar.activation(out=warm, in_=eps_t, func=AF.Sqrt,
                             bias=eps_t, scale=1.0)

        def row_bcast(nm, width):
            row = const.tile([1, width], fp32, tag=f"r_{nm}", name=f"r_{nm}")
            nc.gpsimd.dma_start(
                out=row, in_=io[nm].rearrange("(a w) -> a w", a=1))
            t = const.tile([P, width], fp32, tag=f"bc_{nm}", name=f"bc_{nm}")
            nc.gpsimd.partition_broadcast(t, row)
            return t

        ones_row = const.tile([1, P], bf16, tag="ones_row", name="ones_row")
        nc.vector.memset(ones_row, 1.0)
        b_v_row = const.tile([1, VW], bf16, tag="bvrow", name="b_v_row")
        nc.gpsimd.dma_start(
            out=b_v_row, in_=io["b_v_aug"].rearrange("(a w) -> a w", a=1))

        # col-bias sources [m, 128] (transposed on PE once tiles land)
        bqk_src = const.tile([2 * CK, P], fp32, tag="bqk_src", name="bqk_src")
        nc.gpsimd.dma_start(
            out=bqk_src, in_=io["b_qk"].rearrange("(o p) -> o p", p=P))
        bfc1_src = const.tile([HK, P], fp32, tag="bfc1_src", name="bfc1_src")
        nc.gpsimd.dma_start(
            out=bfc1_src, in_=io["b_fc1"].rearrange("(o p) -> o p", p=P))
        b_qk_col = const.tile([P, 2 * CK], fp32, tag="bqk_col", name="b_qk_col")
        b_fc1_col = const.tile([P, HK], fp32, tag="bfc1_col", name="b_fc1_col")

        # h2T: one [P, CK, N] tile -> LN2 transposes land with ONE wide copy
        h2T = h2Tpool.tile([P, CK, N], bf16, tag="h2T", name="h2T")

        w_proj_sb = []
        for k in range(CK):
            wt = wprojp.tile([P, C], bf16, tag=f"wp_{k}", name=f"wp_{k}")
            w_proj_sb.append(wt)

        # ---------------- LN helper (scalar-engine stats) ----------------
        def ln_stats_hn(src, sum_in=None):
            """mean/var stats split across Scalar/DVE; hn on gpsimd."""
            if sum_in is None:
                scr = work.tile([P, C], f8, tag="scr", name="scr", bufs=1)
                mean = work.tile([P, 1], fp32, tag="s1", name="s1")
                nc.scalar.activation(out=scr, in_=src, func=AF.Copy,
                                     scale=1.0 / C, accum_out=mean)
                # sum of squares on DVE (stt accum), scaled later
                scr2 = work.tile([P, C], fp32, tag="scrv", name="scrv",
                                 bufs=1)
                s2r = work.tile([P, 1], fp32, tag="s2", name="s2")
                nc.vector.scalar_tensor_tensor(
                    out=scr2, in0=src, scalar=1.0, in1=src,
                    op0=ALU.mult, op1=ALU.mult, accum_out=s2r)
                s2scale = 1.0 / C
            else:
                mean = work.tile([P, 1], fp32, tag="s1", name="s1")
                nc.vector.tensor_scalar(out=mean, in0=sum_in,
                                        scalar1=1.0 / C, scalar2=None,
                                        op0=ALU.mult)
                scr2 = work.tile([P, C], f8, tag="scr", name="scr2", bufs=1)
                s2r = work.tile([P, 1], fp32, tag="s2", name="s2")
                nc.scalar.activation(out=scr2, in_=src, func=AF.Square,
                                     scale=(1.0 / C) ** 0.5, accum_out=s2r)
                s2scale = 1.0
            m2 = work.tile([P, 1], fp32, tag="m2", name="m2")
            nc.vector.tensor_mul(out=m2, in0=mean, in1=mean)
            vv = work.tile([P, 1], fp32, tag="vv", name="vv")
            nc.vector.scalar_tensor_tensor(
                out=vv, in0=s2r, scalar=s2scale, in1=m2,
                op0=ALU.mult, op1=ALU.subtract)
            std = work.tile([P, 1], fp32, tag="std", name="std")
            nc.scalar.activation(out=std, in_=vv, func=AF.Sqrt,
                                 bias=eps_t, scale=1.0)
            istd = work.tile([P, 1], fp32, tag="istd", name="istd")
            nc.vector.reciprocal(out=istd, in_=std)
            hn = work.tile([P, C], bf16, tag="hn", name="hn")
            nc.vector.tensor_scalar(out=hn, in0=src,
                                    scalar1=mean, scalar2=istd,
                                    op0=ALU.subtract, op1=ALU.mult)
            return hn

        def transpose_tile(hn, psTpool, dstT, t, eng):
            """6 transposes into one psum bank + one wide strided copy."""
            pt6 = psTpool.tile([P, C], bf16, tag="tr", name="tr")
            for c in range(CK):
                nc.tensor.transpose(pt6[:, c * P:(c + 1) * P],
                                    hn[:, c * P:(c + 1) * P], ident)
            half = CK // 2
            src = pt6.rearrange("p (c q) -> p c q", c=CK)
            dst_lo = dstT[:, :half, t * P:(t + 1) * P]
            dst_hi = dstT[:, half:, t * P:(t + 1) * P]
            if eng == "s":
                nc.scalar.copy(out=dst_lo, in_=src[:, :half, :])
                nc.vector.tensor_copy(out=dst_hi, in_=src[:, half:, :])
            else:
                nc.vector.tensor_copy(out=dst_lo, in_=src[:, :half, :])
                nc.scalar.copy(out=dst_hi, in_=src[:, half:, :])

        with tc.tile_pool(name="aopool", bufs=1) as aopool:
            attn_oT = [aopool.tile([P, N], bf16, tag=f"aoT_{c}",
                                   name=f"aoT_{c}") for c in range(CK)]

            with tc.tile_pool(name="wqk", bufs=1) as wqk_pool, \
                 tc.tile_pool(name="h1Tpool", bufs=1) as h1Tpool, \
                 tc.tile_pool(name="vpool", bufs=1) as vpool:

                w_qk_sb = []
                for j in range(CK // 2):
                    wt = wqk_pool.tile([P, 2, 2 * C], f8, tag=f"wqk_{j}",
                                       name=f"wqk_{j}")
                    w_qk_sb.append(wt)
                h1T = h1Tpool.tile([P, CK, N], f8, tag="h1T", name="h1T")
                v_sb = vpool.tile([P, NT, VW], f8, tag="v_sb", name="v_sb")

                # ---------- phase 1: LN1 + V ----------
                with tc.tile_pool(name="wv", bufs=1) as wv_pool, \
                     tc.tile_pool(name="psT", bufs=2, space="PSUM") as psT:
                    w_v_sb = []
                    for j in range(CK // 2):
                        wt = wv_pool.tile([P, 2, VW], f8, tag=f"wv_{j}",
                                          name=f"wv_{j}")
                        nc.gpsimd.dma_start(
                            out=wt,
                            in_=io["w_v_aug"][2 * j * P:(2 * j + 2) * P,
                                              :].rearrange(
                                "(i p) c -> p i c", p=P))
                        w_v_sb.append(wt)

                    # col-bias transposes (PE idles here anyway)
                    ptb = psT.tile([P, HK], fp32, tag="tr", name="ptb")
                    nc.tensor.transpose(ptb[:, :2 * CK], bqk_src,
                                        ident32[:2 * CK, :2 * CK])
                    nc.vector.tensor_copy(out=b_qk_col, in_=ptb[:, :2 * CK])
                    ptb2 = psT.tile([P, HK], fp32, tag="tr", name="ptb2")
                    nc.tensor.transpose(ptb2, bfc1_src, ident32)
                    nc.vector.tensor_copy(out=b_fc1_col, in_=ptb2)

                    for t in range(NT):
                        hn = ln_stats_hn(x_sb[t])
                        transpose_tile(hn, psT, h1T, t,
                                       "s" if t % 2 else "v")
                        ps = psM.tile([P, 2 * NSL], fp32, tag="mm",
                                      name="mm")
                        for off, w in ((0, NSL), (NSL, VW - NSL)):
                            nc.tensor.matmul(
                                ps[:, off:off + w], ones_row,
                                b_v_row[:, off:off + w],
                                start=True, stop=False)
                            for j in range(CK // 2):
                                nc.tensor.matmul(
                                    ps[:, off:off + w],
                                    h1T[:, 2 * j:2 * j + 2,
                                        t * P:(t + 1) * P],
                                    w_v_sb[j][:, :, off:off + w],
                                    start=False, stop=(j == CK // 2 - 1),
                                    perf_mode=DR)
                        nc.scalar.copy(out=v_sb[:, t, :VW // 2],
                                       in_=ps[:, :VW // 2])
                        nc.vector.tensor_copy(out=v_sb[:, t, VW // 2:VW],
                                              in_=ps[:, VW // 2:VW])
                        if 3 <= t < 3 + CK // 2:
                            # wqk transfers deferred so x tiles keep full
                            # DMA bandwidth: pace the gpsimd queue with a
                            # tiny dep on this tile's v output first
                            j = t - 3
                            pace = work.tile([1, 1], f8, tag="pace",
                                             name="pace")
                            nc.gpsimd.tensor_copy(
                                out=pace, in_=v_sb[0:1, t, 0:1])
                            nc.gpsimd.dma_start(
                                out=w_qk_sb[j],
                                in_=io["w_qk"][2 * j * P:(2 * j + 2) * P,
                                               :].rearrange(
                                    "(i p) c -> p i c", p=P))

                # ---------- phase 2+3: attention ----------
                b_proj_bc = row_bcast("b_proj", C)
                for k in range(CK):
                    nc.sync.dma_start(
                        out=w_proj_sb[k],
                        in_=io["w_proj"][k * P:(k + 1) * P, :])
                with tc.tile_pool(name="kqpool", bufs=2) as kqpool, \
                     tc.tile_pool(name="epool", bufs=3) as epool, \
                     tc.tile_pool(name="rpool", bufs=4) as rpool, \
                     tc.tile_pool(name="psV", bufs=2, space="PSUM") as psV:

                    def emit_k(hp):
                        ps = psM.tile([P, N], fp32, tag="mm", name="mm")
                        for ns in range(NSLABS):
                            sl = slice(ns * NSL, (ns + 1) * NSL)
                            for j in range(CK // 2):
                                nc.tensor.matmul(
                                    ps[:, sl],
                                    w_qk_sb[j][:, :, C + hp * P:
                                               C + (hp + 1) * P],
                                    h1T[:, 2 * j:2 * j + 2, sl],
                                    start=(j == 0), stop=(j == CK // 2 - 1),
                                    perf_mode=DR)
                        kt = kqpool.tile([P, N], bf16, tag="ksb", name="ksb")
                        nc.vector.tensor_scalar(
                            out=kt, in0=ps,
                            scalar1=b_qk_col[:, CK + hp:CK + hp + 1],
                            scalar2=None, op0=ALU.add)
                        return kt

                    def emit_q(hp, ns):
                        ps = psM.tile([P, N], fp32, tag="mm", name="mm")
                        sl = slice(ns * NSL, (ns + 1) * NSL)
                        for j in range(CK // 2):
                            nc.tensor.matmul(
                                ps[:, :NSL],
                                w_qk_sb[j][:, :, hp * P:(hp + 1) * P],
                                h1T[:, 2 * j:2 * j + 2, sl],
                                start=(j == 0), stop=(j == CK // 2 - 1),
                                perf_mode=DR)
                        qt = kqpool.tile([P, NSL], bf16, tag="qsb",
                                         name="qsb")
                        nc.scalar.activation(
                            out=qt, in_=ps[:, :NSL], func=AF.Identity,
                            bias=b_qk_col[:, hp:hp + 1], scale=1.0)
                        return qt

                    def emit_scores_half(kt, qt, half, E):
                        pr = slice(half * D, (half + 1) * D)
                        for j in range(4):
                            ps = psM.tile([P, N], fp32, tag="mm",
                                          name="mm")
                            for i in range(2):
                                mt = 2 * j + i
                                nc.tensor.matmul(
                                    ps[:, i * NSL:(i + 1) * NSL],
                                    kt[pr, mt * P:(mt + 1) * P],
                                    qt[pr, :],
                                    start=True, stop=True)
                            e = epool.tile([P, N], f8,
                                           tag=f"E_{half}_{j}",
                                           name=f"E_{half}_{j}")
                            nc.scalar.activation(out=e, in_=ps,
                                                 func=AF.Exp,
                                                 scale=0.125)
                            E[(half, j)] = e

                    def emit_attnv_half(hp, ns, E, half):
                        nsl = slice(ns * NSL, (ns + 1) * NSL)
                        if True:
                            h = 2 * hp + half
                            ps_o = psV.tile([P, NSL], fp32, tag="vo",
                                            name="vo")[:D + 1, :]
                            for u in range(NT // 2):
                                e = E[(half, u)]
                                nc.tensor.matmul(
                                    ps_o,
                                    v_sb[:, 2 * u:2 * u + 2,
                                         h * VB:h * VB + D + 1],
                                    e.rearrange("p (i n) -> p i n", i=2),
                                    start=(u == 0), stop=(u == NT // 2 - 1),
                                    perf_mode=DR)
                            # custom-DVE fast reciprocal is 5x cheaper than
                            # InstReciprocal but computes garbage for
                            # nonzero partition bases (verified: NaN), so
                            # stage den from partition 64 to 0 first
                            dn = rpool.tile([1, NSL], fp32, tag="dn",
                                            name="dn")
                            nc.vector.tensor_copy(
                                out=dn, in_=ps_o[D:D + 1, :])
                            r = rpool.tile([1, NSL], fp32, tag="r",
                                           name="r")
                            nc.vector.reciprocal_approx_fast(
                                out=r, in_=dn)
                            rb = rpool.tile([D, NSL], fp32, tag="rb",
                                            name="rb")
                            nc.gpsimd.partition_broadcast(rb, r)
                            nc.vector.tensor_mul(
                                out=attn_oT[hp][half * D:(half + 1) * D,
                                                nsl],
                                in0=ps_o[:D, :], in1=rb)

                    prev = None
                    x2q = list(range(NT))  # b_proj pre-add (in-place x)
                    kt_cur = emit_k(0)
                    kt_next = None
                    for hp in range(HPAIRS):
                        for ns in range(NSLABS):
                            qt = emit_q(hp, ns)
                            if ns == 0 and hp + 1 < HPAIRS:
                                kt_next = emit_k(hp + 1)
                            E = {}
                            emit_scores_half(kt_cur, qt, 0, E)
                            if prev is not None:
                                emit_attnv_half(*prev, 0)
                            emit_scores_half(kt_cur, qt, 1, E)
                            if prev is not None:
                                emit_attnv_half(*prev, 1)
                            if x2q:
                                t = x2q.pop(0)
                                nc.vector.tensor_add(
                                    out=x_sb[t], in0=x_sb[t],
                                    in1=b_proj_bc)
                            prev = (hp, ns, E)
                        kt_cur = kt_next
                    emit_attnv_half(*prev, 0)
                    emit_attnv_half(*prev, 1)

            # ---------- phase 4+5+6: proj + LN2 (lagged) + MLP ----------
            b_fc2_bc = row_bcast("b_fc2", C)
            with tc.tile_pool(name="wfc1", bufs=1) as w1pool, \
                 tc.tile_pool(name="wfc2", bufs=1) as w2pool, \
                 tc.tile_pool(name="psT2", bufs=2, space="PSUM") as psT2:
                w1 = []
                for k in range(CK):
                    wt = w1pool.tile([P, HID], bf16, tag=f"wfc1_{k}",
                                     name=f"wfc1_{k}")
                    nc.sync.dma_start(
                        out=wt, in_=io["w_fc1"][k * P:(k + 1) * P, :])
                    w1.append(wt)
                w2g = []
                for g in range(CK):
                    wt = w2pool.tile([P, HK // CK, C], bf16, tag=f"wfc2_{g}",
                                     name=f"wfc2_{g}")
                    nc.sync.dma_start(
                        out=wt,
                        in_=io["w_fc2"][g * 512:(g + 1) * 512, :].rearrange(
                            "(o p) c -> p o c", p=P))
                    w2g.append(wt)

                def emit_proj(t):
                    ps = psM.tile([P, 2 * NSL], fp32, tag="mm", name="mm")
                    for off, w in ((0, NSL), (NSL, C - NSL)):
                        for k in range(CK):
                            nc.tensor.matmul(
                                ps[:, off:off + w],
                                attn_oT[k][:, t * P:(t + 1) * P],
                                w_proj_sb[k][:, off:off + w],
                                start=(k == 0), stop=(k == CK - 1))
                    s1raw = work.tile([P, 1], fp32, tag="s1r", name="s1r")
                    nc.vector.scalar_tensor_tensor(
                        out=x_sb[t], in0=ps[:, :C], scalar=1.0,
                        in1=x_sb[t], op0=ALU.mult, op1=ALU.add,
                        accum_out=s1raw)
                    return s1raw

                def emit_ln2(t, s1raw):
                    hn = ln_stats_hn(x_sb[t], sum_in=s1raw)
                    transpose_tile(hn, psT2, h2T, t, "s" if t % 2 else "v")

                LAG = 2
                s1s = {}
                for t in range(NT):
                    s1s[t] = emit_proj(t)
                    if t >= LAG:
                        emit_ln2(t - LAG, s1s.pop(t - LAG))

                gelu_f = AF.Gelu if gelu_mode == "hw" else AF.Identity
                with tc.tile_pool(name="gpool", bufs=1) as gpool, \
                     tc.tile_pool(name="opool", bufs=2) as opool:
                    gT = gpool.tile([P, HK, N], bf16, tag="gT", name="gT")
                    x2q = list(range(NT))  # b_fc2 pre-add (in-place x)

                    def emit_fc1(ns, ms):
                        sl = slice(ns * NSL, (ns + 1) * NSL)
                        for m in ms:
                            ps = psM.tile([P, 2 * NSL], fp32, tag="mm",
                                          name="mm")
                            for k in range(CK):
                                nc.tensor.matmul(
                                    ps[:, :NSL],
                                    w1[k][:, m * P:(m + 1) * P],
                                    h2T[:, k, sl],
                                    start=(k == 0), stop=(k == CK - 1))
                            nc.scalar.activation(
                                out=gT[:, m, sl], in_=ps[:, :NSL],
                                func=gelu_f,
                                bias=b_fc1_col[:, m:m + 1], scale=1.0)

                    # LN2 tail FIRST (all Sqrts contiguous -> one act-table
                    # load), then the fc1 gelus (one gelu table load); the
                    # old interleave thrashed 5 act-table loads (~13us of
                    # Scalar time in the fc1-slab-0 window)
                    for t in range(NT - LAG, NT):
                        emit_ln2(t, s1s.pop(t))
                    for i in range(4):
                        emit_fc1(0, range(i * 6, (i + 1) * 6))
                        if x2q:
                            tt = x2q.pop(0)
                            nc.gpsimd.tensor_add(
                                out=x_sb[tt], in0=x_sb[tt], in1=b_fc2_bc)
                    emit_fc1(1, range(HK))
                    while x2q:
                        tt = x2q.pop(0)
                        nc.vector.tensor_add(
                            out=x_sb[tt], in0=x_sb[tt], in1=b_fc2_bc)

                    for t in range(NT):
                        ps = psM.tile([P, 2 * NSL], fp32, tag="mm",
                                      name="mm")
                        ot = opool.tile([P, C], fp32, tag="ot", name="ot")
                        for off, w in ((0, NSL), (NSL, C - NSL)):
                            for k in range(HK):
                                nc.tensor.matmul(
                                    ps[:, off:off + w],
                                    gT[:, k, t * P:(t + 1) * P],
                                    w2g[k // 4][:, k % 4, off:off + w],
                                    start=(k == 0), stop=(k == HK - 1))
                            nc.vector.tensor_add(
                                out=ot[:, off:off + w],
                                in0=ps[:, off:off + w],
                                in1=x_sb[t][:, off:off + w])
                            nc.sync.dma_start(
                                out=io["out"][t * P:(t + 1) * P,
                                              off:off + w],
                                in_=ot[:, off:off + w])


def build_program(gelu_mode="hw", mm_dt="bf16"):
    import concourse.tile as tile
    from concourse import bacc, mybir

    fp32 = mybir.dt.float32
    bf16 = mybir.dt.bfloat16
    nc = bacc.Bacc("TRN2", target_bir_lowering=False, debug=False,
                   num_devices=8)

    f8 = mybir.dt.float8e4
    shapes = {
        "x": ([N, C], fp32),
        "w_qk": ([C, 2 * C], f8), "b_qk": ([2 * C], fp32),
        "w_v_aug": ([C, VW], f8), "b_v_aug": ([VW], fp32),
        "w_proj": ([C, C], bf16), "b_proj": ([C], fp32),
        "w_fc1": ([C, HID], bf16), "b_fc1": ([HID], fp32),
        "w_fc2": ([HID, C], bf16), "b_fc2": ([C], fp32),
    }
    io = {}
    for name, (shp, dt) in shapes.items():
        io[name] = nc.dram_tensor(name, shp, dt, kind="ExternalInput").ap()
    io["out"] = nc.dram_tensor("out", [N, C], fp32, kind="ExternalOutput").ap()

    with tile.TileContext(nc) as tc:
        _emit(tc, io, gelu_mode=gelu_mode, mm_dt=mm_dt)
    nc.compile()
    return nc


def host_prep(inputs, mm_dt="bf16"):
    """Fold LN1/LN2 scale+shift into w_qk/w_v/w_fc1; build v-aug layout;
    cast weights to bf16."""
    import ml_dtypes
    f32 = np.float32
    bf = ml_dtypes.bfloat16

    x = np.asarray(inputs["x"], f32)
    w_qkv = np.asarray(inputs["w_qkv"], f32)
    b_qkv = np.asarray(inputs["b_qkv"], f32)
    ln1_w = np.asarray(inputs["ln1_w"], f32)
    ln1_b = np.asarray(inputs["ln1_b"], f32)
    ln2_w = np.asarray(inputs["ln2_w"], f32)
    ln2_b = np.asarray(inputs["ln2_b"], f32)

    w_q = w_qkv[:, 0:C]
    w_k = w_qkv[:, C:2 * C]
    w_v = w_qkv[:, 2 * C:3 * C]
    b_q = b_qkv[0:C]
    b_k = b_qkv[C:2 * C]
    b_v = b_qkv[2 * C:3 * C]

    # fold LN1: h = hn*ln1_w + ln1_b  =>  W' = ln1_w[:,None]*W, b' = b + W.T@ln1_b
    w_qk = np.concatenate([w_q, w_k], axis=1)          # [C, 2C]
    w_qk_f = ln1_w[:, None] * w_qk
    b_qk_f = np.concatenate([b_q, b_k]) + w_qk.T @ ln1_b

    w_v_f = ln1_w[:, None] * w_v
    b_v_f = b_v + w_v.T @ ln1_b
    w_v_aug = np.zeros((C, VW), f32)
    b_v_aug = np.zeros((VW,), f32)
    for h in range(H):
        w_v_aug[:, h * VB:h * VB + D] = w_v_f[:, h * D:(h + 1) * D]
        b_v_aug[h * VB:h * VB + D] = b_v_f[h * D:(h + 1) * D]
        b_v_aug[h * VB + D] = 1.0

    w_fc1 = np.asarray(inputs["w_fc1"], f32)
    b_fc1 = np.asarray(inputs["b_fc1"], f32)
    w_fc1_f = ln2_w[:, None] * w_fc1
    b_fc1_f = b_fc1 + w_fc1.T @ ln2_b

    def f8(a):
        # TRN e4m3 matches OCP e4m3fn bit-for-bit for |x| <= 240
        return np.ascontiguousarray(
            np.clip(a, -240, 240)).astype(ml_dtypes.float8_e4m3)

    common = {
        "w_qk": f8(w_qk_f),
        "b_qk": np.ascontiguousarray(b_qk_f, f32),
        "w_v_aug": f8(w_v_aug),
        "b_v_aug": b_v_aug,
        "w_proj": np.ascontiguousarray(
            np.asarray(inputs["w_proj"], f32)).astype(bf),
        "b_proj": np.ascontiguousarray(np.asarray(inputs["b_proj"], f32)),
        "w_fc1": np.ascontiguousarray(w_fc1_f).astype(bf),
        "b_fc1": np.ascontiguousarray(b_fc1_f, f32),
        "w_fc2": np.ascontiguousarray(np.asarray(inputs["w_fc2"], f32)).astype(bf),
        "b_fc2": np.ascontiguousarray(np.asarray(inputs["b_fc2"], f32)),
    }
    in_maps = []
    for i in range(x.shape[0]):
        m = dict(common)
        m["x"] = np.ascontiguousarray(x[i])
        in_maps.append(m)
    return in_maps


_CACHE = {}


def kernel(**inputs):
    from concourse.bass_utils import run_bass_kernel_spmd

    if "nc" not in _CACHE:
        _CACHE["nc"] = build_program(gelu_mode="hw")
    nc = _CACHE["nc"]
    in_maps = host_prep(inputs)
    res = run_bass_kernel_spmd(nc, in_maps, list(range(8)))
    out = np.stack([r["out"] for r in res.results], axis=0)
    return out.astype(np.float32)

